# revision 10
# baseline (speedup 1.0000x reference)
"""DeepInfoMax loss kernel for 8 Trainium2 NeuronCores.

Strategy (hardcoded for B=8192, d=1024, n=16):
  - Data-parallel over batch: core c gets rows [c*1024, (c+1)*1024), plus ONE
    overlap row ((c+1)*1024 % B) of M so the global roll (M_prime) is exact.
  - Activations are kept feature-major ([features, batch]) on-chip so weights
    are the stationary matmul operand.
  - Algebraic sharing: net(M) (global discriminator's M-branch) and the
    y-contribution of the local experts' first layer commute with the batch
    roll, so both are computed ONCE and re-sliced for the joint/marginal pass.
  - bf16 matmuls with fp32 PSUM accumulation; softplus = ln(1+exp(x)) on the
    scalar engine with fused accumulation into per-core partial sums.
  - Host combines 8x [128,8] partial-sum tiles into the final scalar.
"""

import numpy as np
import ml_dtypes

B = 8192
D = 1024
NI = 16
DN = D // NI  # 64
NC = 8
BS = B // NC  # 1024
BSP = BS + 1  # 1025 (overlap col for the exact roll)
ALPHA = 0.5
BETA = 1.0

# column chunks over the 1025-wide (producer) and 1024-wide (consumer) phases
CH_P = [(0, 342), (342, 342), (684, 341)]
CH_C = [(0, 512), (512, 512)]

BF = ml_dtypes.bfloat16

_RUNNER = None  # cached (nc, run) so repeated kernel() calls don't rebuild


def _build_nc():
    import concourse.bass as bass
    import concourse.tile as tile
    import concourse.mybir as mybir
    from concourse import bacc
    from contextlib import ExitStack

    bf = mybir.dt.bfloat16
    f32 = mybir.dt.float32
    AF = mybir.ActivationFunctionType
    OP = mybir.AluOpType

    nc = bacc.Bacc()

    # ---- DRAM I/O ----
    yt3 = nc.dram_tensor("yt3", [8, 128, BS], bf, kind="ExternalInput")
    mt3 = nc.dram_tensor("mt3", [8, 128, BSP], bf, kind="ExternalInput")
    m3p = nc.dram_tensor("m3p", [8, 128, BSP], bf, kind="ExternalInput")
    gw0p = nc.dram_tensor("gw0p", [128, 8 * D], bf, kind="ExternalInput")
    gw1p = nc.dram_tensor("gw1p", [128, 8 * D], bf, kind="ExternalInput")
    bxp = nc.dram_tensor("bxp", [128, 8 * 2176], bf, kind="ExternalInput")
    acatp = nc.dram_tensor("acatp", [128, 2048], bf, kind="ExternalInput")
    w2sp = nc.dram_tensor("w2sp", [128, 2048], bf, kind="ExternalInput")
    w3sp = nc.dram_tensor("w3sp", [128, NI], bf, kind="ExternalInput")
    l0whp = nc.dram_tensor("l0whp", [128, 1024], bf, kind="ExternalInput")
    l1wp = nc.dram_tensor("l1wp", [128, 128], bf, kind="ExternalInput")
    l2wp = nc.dram_tensor("l2wp", [128, 1], bf, kind="ExternalInput")
    gb0c = nc.dram_tensor("gb0c", [128, 8], f32, kind="ExternalInput")
    gb1c = nc.dram_tensor("gb1c", [128, 8], f32, kind="ExternalInput")
    lb1c = nc.dram_tensor("lb1c", [128, NI], f32, kind="ExternalInput")
    lb2c = nc.dram_tensor("lb2c", [128, NI], f32, kind="ExternalInput")
    onesr = nc.dram_tensor("onesr", [1, 128], bf, kind="ExternalInput")
    b3r8 = nc.dram_tensor("b3r8", [1, 128], bf, kind="ExternalInput")
    l0bc = nc.dram_tensor("l0bc", [128, 1], f32, kind="ExternalInput")
    l1bc = nc.dram_tensor("l1bc", [128, 1], f32, kind="ExternalInput")
    l2bc2 = nc.dram_tensor("l2bc2", [128, 2], f32, kind="ExternalInput")
    acc = nc.dram_tensor("acc", [128, 8], f32, kind="ExternalOutput")

    with tile.TileContext(nc) as tc, ExitStack() as ctx:
        pconst = ctx.enter_context(tc.tile_pool(name="const", bufs=1))
        pgw = ctx.enter_context(tc.tile_pool(name="gw", bufs=2))
        pbx = ctx.enter_context(tc.tile_pool(name="bx", bufs=1))
        pac = ctx.enter_context(tc.tile_pool(name="ac", bufs=1))
        pyt = ctx.enter_context(tc.tile_pool(name="yt", bufs=8))
        p25 = ctx.enter_context(tc.tile_pool(name="t25", bufs=16))
        pma = ctx.enter_context(tc.tile_pool(name="ma", bufs=2))
        pm3 = ctx.enter_context(tc.tile_pool(name="m3", bufs=2))
        pyp = ctx.enter_context(tc.tile_pool(name="yp", bufs=6))
        pgy = ctx.enter_context(tc.tile_pool(name="gy", bufs=1))
        ptr4 = ctx.enter_context(tc.tile_pool(name="tr4", bufs=4))
        ptr2 = ctx.enter_context(tc.tile_pool(name="tr2", bufs=2))
        ptr1 = ctx.enter_context(tc.tile_pool(name="tr1", bufs=1))
        ppm = ctx.enter_context(tc.tile_pool(name="pm", bufs=4, space="PSUM"))
        ppp = ctx.enter_context(tc.tile_pool(name="pp", bufs=1, space="PSUM"))

        # ---- constants into SBUF ----
        def cload(dram, shape, dt):
            t = pconst.tile(shape, dt, tag=dram.name, name=dram.name + "_sb")
            nc.sync.dma_start(t[:], dram[:])
            return t

        w3s_sb = cload(w3sp, [128, NI], bf)
        l1w_sb = cload(l1wp, [128, 128], bf)
        l2w_sb = cload(l2wp, [128, 1], bf)
        l0wh_sb = cload(l0whp, [128, 1024], bf)
        w2s_sb = pac.tile([128, 2048], bf, tag="w2s")
        nc.sync.dma_start(w2s_sb[:], w2sp[:])
        gb0_sb = cload(gb0c, [128, 8], f32)
        gb1_sb = cload(gb1c, [128, 8], f32)
        lb1_sb = cload(lb1c, [128, NI], f32)
        lb2_sb = cload(lb2c, [128, NI], f32)
        ones_sb = cload(onesr, [1, 128], bf)
        b3r8_sb = cload(b3r8, [1, 128], bf)
        l0b_sb = cload(l0bc, [128, 1], f32)
        l1b_sb = cload(l1bc, [128, 1], f32)
        l2b_sb = cload(l2bc2, [128, 2], f32)
        acc_sb = pconst.tile([128, 8], f32, tag="acc")
        nc.vector.memset(acc_sb[:], 0.0)

        # ---- phase A inputs ----
        gw0_sb = pgw.tile([128, 8 * D], bf, tag="gw")
        nc.sync.dma_start(gw0_sb[:], gw0p[:])
        mt_sb = []
        for k in range(8):
            t = p25.tile([128, BSP], bf, tag="t25", name=f"mt_{k}")
            nc.sync.dma_start(t[:], mt3[k, :, :])
            mt_sb.append(t)

        # ---- phase A: h_g = relu(M @ gw0 + gb0), feature-major, 1025 cols ----
        hg_sb = []
        for m in range(8):
            t = p25.tile([128, BSP], bf, tag="t25", name=f"hg_{m}")
            hg_sb.append(t)
        for m in range(8):
            for (c0, cw) in CH_P:
                ps = ppm.tile([128, 512], f32, tag="pm")
                for k in range(8):
                    nc.tensor.matmul(
                        ps[:, :cw],
                        gw0_sb[:, k * D + m * 128:k * D + (m + 1) * 128],
                        mt_sb[k][:, c0:c0 + cw],
                        start=(k == 0), stop=(k == 7),
                    )
                nc.scalar.activation(
                    hg_sb[m][:, c0:c0 + cw], ps[:, :cw], AF.Relu,
                    bias=gb0_sb[:, m:m + 1],
                )

        # prefetch gw1
        gw1_sb = pgw.tile([128, 8 * D], bf, tag="gw")
        nc.sync.dma_start(gw1_sb[:], gw1p[:])

        # ---- phase B: hM = h_g @ gw1 + gb1 (no relu), 1025 cols ----
        hm_sb = []
        for m in range(8):
            t = p25.tile([128, BSP], bf, tag="t25", name=f"hm_{m}")
            hm_sb.append(t)
        for m in range(8):
            for (c0, cw) in CH_P:
                ps = ppm.tile([128, 512], f32, tag="pm")
                for k in range(8):
                    nc.tensor.matmul(
                        ps[:, :cw],
                        gw1_sb[:, k * D + m * 128:k * D + (m + 1) * 128],
                        hg_sb[k][:, c0:c0 + cw],
                        start=(k == 0), stop=(k == 7),
                    )
                nc.scalar.activation(
                    hm_sb[m][:, c0:c0 + cw], ps[:, :cw], AF.Identity,
                    bias=gb1_sb[:, m:m + 1],
                )

        # prefetch phase C inputs
        bx_sb = pbx.tile([128, 8 * 2176], bf, tag="bx")
        nc.sync.dma_start(bx_sb[:], bxp[:])
        yt_sb = []
        for k in range(8):
            t = pyt.tile([128, BS], bf, tag="yt", name=f"yt_{k}")
            nc.sync.dma_start(t[:], yt3[k, :, :])
            yt_sb.append(t)
        acat_sb = pac.tile([128, 2048], bf, tag="acat")
        nc.sync.dma_start(acat_sb[:], acatp[:])

        # ---- phase C: y_part (m 0..15) and gy (m 16), 1024 cols ----
        # yp[m] = (y @ Bcat)[:, m-block]^T ; gy = y @ l0w[:1024] + l0b (fp32)
        gy_sb = pgy.tile([128, BS], f32, tag="gy")
        yp_sb = [None] * 16

        def emit_C_m(m):
            if m < 16:
                yp_sb[m] = pyp.tile([128, BS], bf, tag="yp", name=f"yp_{m}")
            for (c0, cw) in CH_C:
                ps = ppm.tile([128, 512], f32, tag="pm")
                for k in range(8):
                    nc.tensor.matmul(
                        ps[:, :cw],
                        bx_sb[:, k * 2176 + m * 128:k * 2176 + (m + 1) * 128],
                        yt_sb[k][:, c0:c0 + cw],
                        start=(k == 0), stop=(k == 7),
                    )
                if m < 16:
                    nc.vector.tensor_copy(yp_sb[m][:, c0:c0 + cw], ps[:, :cw])
                else:
                    nc.scalar.activation(
                        gy_sb[:, c0:c0 + cw], ps[:, :cw], AF.Identity,
                        bias=l0b_sb[:, 0:1],
                    )

        emit_C_m(16)  # gy first (needed only in F, but frees nothing later)
        for m in range(4):
            emit_C_m(m)

        # local scores: psum_p[p][:, e*8+bt] = s(batch bt*128+row, expert e)
        psum_p = [ppp.tile([128, 128], f32, tag=f"pp{p}", name=f"psum_p{p}")
                  for p in range(2)]
        psum_g = ppp.tile([128, 16], f32, tag="pg", name="psum_g")

        # ---- phase D+E interleaved per expert ----
        m3_sb = [None] * 8
        for e in range(NI):
            # emit C for expert e+4 lazily: keeps the yp pool small and
            # overlaps the remaining y_part matmuls with the expert phase
            if e + 4 < 16 and yp_sb[e + 4] is None:
                emit_C_m(e + 4)

            t = e // 2
            po = 64 * (e % 2)
            if m3_sb[t] is None:
                m3t = pm3.tile([128, BSP], bf, tag="m3", name=f"m3_{t}")
                nc.sync.dma_start(m3t[:], m3p[t, :, :])
                m3_sb[t] = m3t

            # D_e: mA_e = M3_e @ A_e + lb1_e  -> [128, 1025] bf16
            ma_t = pma.tile([128, BSP], bf, tag="ma")
            for (c0, cw) in CH_P:
                ps = ppm.tile([128, 512], f32, tag="pm")
                nc.tensor.matmul(
                    ps[:, :cw],
                    acat_sb[po:po + 64, e * 128:(e + 1) * 128],
                    m3_sb[t][po:po + 64, c0:c0 + cw],
                    start=True, stop=True,
                )
                nc.vector.tensor_scalar_add(
                    ma_t[:, c0:c0 + cw], ps[:, :cw], lb1_sb[:, e:e + 1])

            # E_e: both passes
            for p in range(2):
                off = p  # joint reads cols 0..1023, marginal cols 1..1024
                z1 = ptr4.tile([128, BS], bf, tag="zh")
                nc.vector.tensor_add(
                    z1[:], ma_t[:, off:off + BS], yp_sb[e][:])
                h1 = ptr4.tile([128, BS], bf, tag="zh")
                nc.vector.tensor_scalar_max(h1[:], z1[:], 0.0)
                h2 = ptr2.tile([128, BS], bf, tag="h2")
                for ci, (c0, cw) in enumerate(CH_C):
                    ps2 = ppm.tile([128, 512], f32, tag="pm")
                    nc.tensor.matmul(
                        ps2[:, :cw],
                        w2s_sb[:, e * 128:(e + 1) * 128],
                        h1[:, c0:c0 + cw],
                        start=True, stop=True,
                    )
                    nc.vector.tensor_scalar(
                        h2[:, c0:c0 + cw], ps2[:, :cw],
                        lb2_sb[:, e:e + 1], 0.0, op0=OP.add, op1=OP.max)
                # L3 transposed: h2 b-tile stationary, w3 col moving;
                # score for (expert e, batch tile bt) -> psum_p col e*8+bt.
                # b3[e] seeded by a K=1 rank-1 matmul (ones x b3r8).
                nc.tensor.matmul(
                    psum_p[p][:, e * 8:(e + 1) * 8],
                    ones_sb[0:1, :],
                    b3r8_sb[0:1, e * 8:(e + 1) * 8],
                    start=True, stop=False, skip_group_check=True,
                )
                for bt in range(8):
                    nc.tensor.matmul(
                        psum_p[p][:, e * 8 + bt:e * 8 + bt + 1],
                        h2[:, bt * 128:(bt + 1) * 128],
                        w3s_sb[:, e:e + 1],
                        start=False, stop=True, skip_group_check=True,
                    )

        # ---- local softplus reduction: acc col p ----
        for p in range(2):
            sgn = -1.0 if p == 0 else 1.0
            exl = ptr1.tile([128, 128], f32, tag="exl", name=f"exl{p}")
            nc.scalar.activation(exl[:], psum_p[p][:], AF.Exp, scale=sgn)
            spl = ptr1.tile([128, 128], f32, tag="spl", name=f"spl{p}")
            nc.scalar.activation(
                spl[:], exl[:], AF.Ln, bias=1.0,
                accum_out=acc_sb[:, p:p + 1])

        # ---- phase F: global discriminator, both passes ----
        for p in range(2):
            off = p
            sgn = -1.0 if p == 0 else 1.0
            for ci, (c0, cw) in enumerate(CH_C):
                ps = ppm.tile([128, 512], f32, tag="pm")
                for k in range(8):
                    nc.tensor.matmul(
                        ps[:, :cw],
                        l0wh_sb[:, k * 128:(k + 1) * 128],
                        hm_sb[k][:, off + c0:off + c0 + cw],
                        start=(k == 0), stop=(k == 7),
                    )
                z0 = ptr2.tile([128, 512], bf, tag="z0")
                nc.vector.scalar_tensor_tensor(
                    z0[:, :cw], ps[:, :cw], 0.0, gy_sb[:, c0:c0 + cw],
                    op0=OP.add, op1=OP.add)
                h0 = ptr2.tile([128, 512], bf, tag="h0")
                nc.scalar.activation(h0[:, :cw], z0[:, :cw], AF.Relu)
                ps1 = ppm.tile([128, 512], f32, tag="pm")
                nc.tensor.matmul(
                    ps1[:, :cw], l1w_sb[:], h0[:, :cw], start=True, stop=True)
                h1g = ptr2.tile([128, 512], bf, tag="h1g")
                nc.scalar.activation(
                    h1g[:, :cw], ps1[:, :cw], AF.Relu, bias=l1b_sb[:, 0:1])
                for bti in range(4):
                    bt = ci * 4 + bti
                    nc.tensor.matmul(
                        psum_g[:, p * 8 + bt:p * 8 + bt + 1],
                        h1g[:, bti * 128:(bti + 1) * 128],
                        l2w_sb[:, 0:1],
                        start=True, stop=True,
                    )

        # ---- global softplus reduction: acc col 2+p ----
        for p in range(2):
            sgn = -1.0 if p == 0 else 1.0
            exg = ptr1.tile([128, 16], f32, tag="exg", name=f"exg{p}")
            nc.scalar.activation(
                exg[:, :8], psum_g[:, p * 8:(p + 1) * 8], AF.Exp,
                scale=sgn, bias=l2b_sb[:, p:p + 1])
            spg = ptr1.tile([128, 16], f32, tag="spg", name=f"spg{p}")
            nc.scalar.activation(
                spg[:, :8], exg[:, :8], AF.Ln, bias=1.0,
                accum_out=acc_sb[:, 2 + p:3 + p])

        # ---- output ----
        nc.sync.dma_start(acc[:], acc_sb[:])

    nc.finalize()
    return nc


def _prep_shared(inputs):
    """Weight repack (identical for all cores), fp32 -> bf16."""
    f32 = np.float32
    gw0 = np.asarray(inputs["gw0"], f32)
    gw1 = np.asarray(inputs["gw1"], f32)
    l0w = np.asarray(inputs["l0w"], f32)
    l1w = np.asarray(inputs["l1w"], f32)
    l2w = np.asarray(inputs["l2w"], f32)
    lW1 = np.asarray(inputs["lW1"], f32)
    lW2 = np.asarray(inputs["lW2"], f32)
    lW3 = np.asarray(inputs["lW3"], f32)
    gb0 = np.asarray(inputs["gb0"], f32)
    gb1 = np.asarray(inputs["gb1"], f32)
    l0b = np.asarray(inputs["l0b"], f32)
    l1b = np.asarray(inputs["l1b"], f32)
    l2b = np.asarray(inputs["l2b"], f32)
    lb1 = np.asarray(inputs["lb1"], f32)
    lb2 = np.asarray(inputs["lb2"], f32)
    lb3 = np.asarray(inputs["lb3"], f32)

    def pk(a, kb):  # [K, N] -> [128, (K/128)*N] col-block k = rows k*128..
        K, N = a.shape
        return np.ascontiguousarray(
            a.reshape(K // 128, 128, N).transpose(1, 0, 2).reshape(128, -1))

    bcatx = np.concatenate(
        [lW1[:, DN:, :].transpose(1, 0, 2).reshape(D, NI * 128), l0w[:D]], axis=1)
    sh = {
        "gw0p": pk(gw0, 128).astype(BF),
        "gw1p": pk(gw1, 128).astype(BF),
        "bxp": pk(bcatx, 128).astype(BF),
        "acatp": np.ascontiguousarray(np.concatenate([
            lW1[:, :DN, :].transpose(1, 0, 2).reshape(DN, NI * 128)] * 2,
            axis=0)).astype(BF),
        "w2sp": np.ascontiguousarray(
            lW2.transpose(1, 0, 2).reshape(128, NI * 128)).astype(BF),
        "w3sp": np.ascontiguousarray(lW3[:, :, 0].T).astype(BF),
        "l0whp": pk(l0w[D:], 128).astype(BF),
        "l1wp": l1w.astype(BF),
        "l2wp": l2w.astype(BF),
        "gb0c": np.ascontiguousarray(gb0.reshape(8, 128).T),
        "gb1c": np.ascontiguousarray(gb1.reshape(8, 128).T),
        "lb1c": np.ascontiguousarray(lb1.T),
        "lb2c": np.ascontiguousarray(lb2.T),
        "onesr": np.ones((1, 128), BF),
        "b3r8": np.repeat(lb3[:, 0], 8)[None, :].astype(BF),
        "l0bc": np.ascontiguousarray(l0b[:, None]),
        "l1bc": np.ascontiguousarray(l1b[:, None]),
        "l2bc2": np.ascontiguousarray(
            np.stack([np.full(128, -l2b[0], f32),
                      np.full(128, l2b[0], f32)], axis=1)),
    }
    return sh


def _prep_core(inputs, c):
    f32 = np.float32
    y = np.asarray(inputs["y"], f32)
    M = np.asarray(inputs["M"], f32)
    r0 = c * BS
    rows = np.arange(r0, r0 + BSP) % B  # 1025 rows incl. overlap
    Ms = M[rows]  # [1025, 1024]
    ys = y[r0:r0 + BS]  # [1024, 1024]
    yt = np.ascontiguousarray(ys.T).astype(BF)  # [1024 feat, 1024]
    mt = np.ascontiguousarray(Ms.T).astype(BF)  # [1024 feat, 1025]
    # expert-major M: m3t[e, p, b] = Ms[b, p*16+e]; packed 2 experts/tile
    m3t = np.ascontiguousarray(
        Ms.reshape(BSP, DN, NI).transpose(2, 1, 0)).astype(BF)  # [16,64,1025]
    return {
        "yt3": np.ascontiguousarray(yt.reshape(8, 128, BS)),
        "mt3": np.ascontiguousarray(mt.reshape(8, 128, BSP)),
        "m3p": np.ascontiguousarray(m3t.reshape(8, 128, BSP)),
    }


def combine_partials(accs):
    """accs: list of 8 [128, 8] fp32 arrays -> scalar loss (float64 math)."""
    a = np.stack([np.asarray(x, np.float64) for x in accs])  # [8,128,8]
    sl_j = a[:, :, 0].sum()
    sl_m = a[:, :, 1].sum()
    sg_j = a[:, :, 2].sum()
    sg_m = a[:, :, 3].sum()
    local = BETA * (sl_m + sl_j) / (B * NI)
    glob = ALPHA * (sg_m + sg_j) / B
    return np.float32(local + glob)


def make_in_maps(inputs):
    sh = _prep_shared(inputs)
    return [dict(sh, **_prep_core(inputs, c)) for c in range(NC)]


def get_runner():
    global _RUNNER
    if _RUNNER is None:
        _RUNNER = _build_nc()
    return _RUNNER


def kernel(**inputs) -> np.ndarray:
    from concourse.bass_utils import run_bass_kernel_spmd

    nc = get_runner()
    in_maps = make_in_maps(inputs)
    res = run_bass_kernel_spmd(nc, in_maps, list(range(NC)))
    return combine_partials([r["acc"] for r in res.results])


# revision 12
# speedup vs baseline: 1.1006x; 1.1006x over previous
"""DeepInfoMax loss kernel for 8 Trainium2 NeuronCores.

Strategy (hardcoded for B=8192, d=1024, n=16):
  - Data-parallel over batch: core c gets rows [c*1024, (c+1)*1024), plus ONE
    overlap row ((c+1)*1024 % B) of M so the global roll (M_prime) is exact.
  - Activations are kept feature-major ([features, batch]) on-chip so weights
    are the stationary matmul operand.
  - Algebraic sharing: net(M) (global discriminator's M-branch) and the
    y-contribution of the local experts' first layer commute with the batch
    roll, so both are computed ONCE and re-sliced for the joint/marginal pass.
  - bf16 matmuls with fp32 PSUM accumulation; softplus = ln(1+exp(x)) on the
    scalar engine with fused accumulation into per-core partial sums.
  - Host combines 8x [128,8] partial-sum tiles into the final scalar.
"""

import numpy as np
import ml_dtypes

B = 8192
D = 1024
NI = 16
DN = D // NI  # 64
NC = 8
BS = B // NC  # 1024
BSP = BS + 1  # 1025 (overlap col for the exact roll)
ALPHA = 0.5
BETA = 1.0

# column chunks over the 1025-wide (producer) and 1024-wide (consumer) phases
CH_P = [(0, 342), (342, 342), (684, 341)]
CH_C = [(0, 512), (512, 512)]

BF = ml_dtypes.bfloat16

_RUNNER = None  # cached (nc, run) so repeated kernel() calls don't rebuild


def _build_nc():
    import concourse.bass as bass
    import concourse.tile as tile
    import concourse.mybir as mybir
    from concourse import bacc
    from contextlib import ExitStack

    bf = mybir.dt.bfloat16
    f32 = mybir.dt.float32
    AF = mybir.ActivationFunctionType
    OP = mybir.AluOpType

    nc = bacc.Bacc()

    # ---- DRAM I/O ----
    yt3 = nc.dram_tensor("yt3", [8, 128, BS], bf, kind="ExternalInput")
    mt3 = nc.dram_tensor("mt3", [8, 128, BSP], bf, kind="ExternalInput")
    m3p = nc.dram_tensor("m3p", [8, 128, BSP], bf, kind="ExternalInput")
    gw0p = nc.dram_tensor("gw0p", [128, 8 * D], bf, kind="ExternalInput")
    gw1p = nc.dram_tensor("gw1p", [128, 8 * D], bf, kind="ExternalInput")
    bxp = nc.dram_tensor("bxp", [128, 8 * 2176], bf, kind="ExternalInput")
    acatp = nc.dram_tensor("acatp", [128, 2048], bf, kind="ExternalInput")
    w2sp = nc.dram_tensor("w2sp", [128, 2048], bf, kind="ExternalInput")
    w3sp = nc.dram_tensor("w3sp", [128, NI], bf, kind="ExternalInput")
    l0whp = nc.dram_tensor("l0whp", [128, 1024], bf, kind="ExternalInput")
    l1wp = nc.dram_tensor("l1wp", [128, 128], bf, kind="ExternalInput")
    l2wp = nc.dram_tensor("l2wp", [128, 1], bf, kind="ExternalInput")
    gb0c = nc.dram_tensor("gb0c", [128, 8], f32, kind="ExternalInput")
    gb1c = nc.dram_tensor("gb1c", [128, 8], f32, kind="ExternalInput")
    lb1c = nc.dram_tensor("lb1c", [128, NI], f32, kind="ExternalInput")
    lb2c = nc.dram_tensor("lb2c", [128, NI], f32, kind="ExternalInput")
    onesr = nc.dram_tensor("onesr", [1, 128], bf, kind="ExternalInput")
    b3r8 = nc.dram_tensor("b3r8", [1, 128], bf, kind="ExternalInput")
    l0bc = nc.dram_tensor("l0bc", [128, 1], f32, kind="ExternalInput")
    l1bc = nc.dram_tensor("l1bc", [128, 1], f32, kind="ExternalInput")
    l2bc2 = nc.dram_tensor("l2bc2", [128, 2], f32, kind="ExternalInput")
    acc = nc.dram_tensor("acc", [128, 8], f32, kind="ExternalOutput")

    with tile.TileContext(nc) as tc, ExitStack() as ctx:
        pconst = ctx.enter_context(tc.tile_pool(name="const", bufs=1))
        pgw = ctx.enter_context(tc.tile_pool(name="gw", bufs=16))
        pbx = ctx.enter_context(tc.tile_pool(name="bx", bufs=8))
        pac = ctx.enter_context(tc.tile_pool(name="ac", bufs=1))
        pyt = ctx.enter_context(tc.tile_pool(name="yt", bufs=8))
        p25 = ctx.enter_context(tc.tile_pool(name="t25", bufs=16))
        pma = ctx.enter_context(tc.tile_pool(name="ma", bufs=2))
        pm3 = ctx.enter_context(tc.tile_pool(name="m3", bufs=2))
        pyp = ctx.enter_context(tc.tile_pool(name="yp", bufs=6))
        pgy = ctx.enter_context(tc.tile_pool(name="gy", bufs=1))
        ptr4 = ctx.enter_context(tc.tile_pool(name="tr4", bufs=4))
        ptr2 = ctx.enter_context(tc.tile_pool(name="tr2", bufs=2))
        ptr1 = ctx.enter_context(tc.tile_pool(name="tr1", bufs=1))
        ppm = ctx.enter_context(tc.tile_pool(name="pm", bufs=4, space="PSUM"))
        ppp = ctx.enter_context(tc.tile_pool(name="pp", bufs=1, space="PSUM"))

        # ---- constants into SBUF ----
        def cload(dram, shape, dt):
            t = pconst.tile(shape, dt, tag=dram.name, name=dram.name + "_sb")
            nc.sync.dma_start(t[:], dram[:])
            return t

        w3s_sb = cload(w3sp, [128, NI], bf)
        l1w_sb = cload(l1wp, [128, 128], bf)
        l2w_sb = cload(l2wp, [128, 1], bf)
        l0wh_sb = cload(l0whp, [128, 1024], bf)
        w2s_sb = pac.tile([128, 2048], bf, tag="w2s")
        nc.sync.dma_start(w2s_sb[:], w2sp[:])
        gb0_sb = cload(gb0c, [128, 8], f32)
        gb1_sb = cload(gb1c, [128, 8], f32)
        lb1_sb = cload(lb1c, [128, NI], f32)
        lb2_sb = cload(lb2c, [128, NI], f32)
        ones_sb = cload(onesr, [1, 128], bf)
        b3r8_sb = cload(b3r8, [1, 128], bf)
        l0b_sb = cload(l0bc, [128, 1], f32)
        l1b_sb = cload(l1bc, [128, 1], f32)
        l2b_sb = cload(l2bc2, [128, 2], f32)
        acc_sb = pconst.tile([128, 8], f32, tag="acc")
        nc.vector.memset(acc_sb[:], 0.0)

        # ---- phase A inputs (k-granular DMAs so compute starts early) ----
        gw0_sb = []
        mt_sb = []
        for k in range(8):
            t = p25.tile([128, BSP], bf, tag="t25", name=f"mt_{k}")
            nc.sync.dma_start(t[:], mt3[k, :, :])
            mt_sb.append(t)
            g = pgw.tile([128, D], bf, tag="gw", name=f"gw0_{k}")
            nc.sync.dma_start(g[:], gw0p[:, k * D:(k + 1) * D])
            gw0_sb.append(g)

        # ---- phase A: h_g = relu(M @ gw0 + gb0), feature-major, 1025 cols ----
        hg_sb = []
        for m in range(8):
            t = p25.tile([128, BSP], bf, tag="t25", name=f"hg_{m}")
            hg_sb.append(t)
        for m in range(8):
            for (c0, cw) in CH_P:
                ps = ppm.tile([128, 512], f32, tag="pm")
                for k in range(8):
                    nc.tensor.matmul(
                        ps[:, :cw],
                        gw0_sb[k][:, m * 128:(m + 1) * 128],
                        mt_sb[k][:, c0:c0 + cw],
                        start=(k == 0), stop=(k == 7),
                    )
                nc.scalar.activation(
                    hg_sb[m][:, c0:c0 + cw], ps[:, :cw], AF.Relu,
                    bias=gb0_sb[:, m:m + 1],
                )

        # prefetch gw1 (k-granular)
        gw1_sb = []
        for k in range(8):
            g = pgw.tile([128, D], bf, tag="gw", name=f"gw1_{k}")
            nc.sync.dma_start(g[:], gw1p[:, k * D:(k + 1) * D])
            gw1_sb.append(g)

        # ---- phase B: hM = h_g @ gw1 + gb1 (no relu), 1025 cols ----
        hm_sb = []
        for m in range(8):
            t = p25.tile([128, BSP], bf, tag="t25", name=f"hm_{m}")
            hm_sb.append(t)
        for m in range(8):
            for (c0, cw) in CH_P:
                ps = ppm.tile([128, 512], f32, tag="pm")
                for k in range(8):
                    nc.tensor.matmul(
                        ps[:, :cw],
                        gw1_sb[k][:, m * 128:(m + 1) * 128],
                        hg_sb[k][:, c0:c0 + cw],
                        start=(k == 0), stop=(k == 7),
                    )
                nc.scalar.activation(
                    hm_sb[m][:, c0:c0 + cw], ps[:, :cw], AF.Identity,
                    bias=gb1_sb[:, m:m + 1],
                )

        # prefetch phase C inputs (k-granular)
        bx_sb = []
        for k in range(8):
            t = pbx.tile([128, 2176], bf, tag="bx", name=f"bx_{k}")
            nc.sync.dma_start(t[:], bxp[:, k * 2176:(k + 1) * 2176])
            bx_sb.append(t)
        yt_sb = []
        for k in range(8):
            t = pyt.tile([128, BS], bf, tag="yt", name=f"yt_{k}")
            nc.sync.dma_start(t[:], yt3[k, :, :])
            yt_sb.append(t)
        acat_sb = pac.tile([128, 2048], bf, tag="acat")
        nc.sync.dma_start(acat_sb[:], acatp[:])

        # ---- phase C: y_part (m 0..15) and gy (m 16), 1024 cols ----
        # yp[m] = (y @ Bcat)[:, m-block]^T ; gy = y @ l0w[:1024] + l0b (fp32)
        gy_sb = pgy.tile([128, BS], f32, tag="gy")
        yp_sb = [None] * 16

        def emit_C_m(m):
            if m < 16:
                yp_sb[m] = pyp.tile([128, BS], bf, tag="yp", name=f"yp_{m}")
            for (c0, cw) in CH_C:
                ps = ppm.tile([128, 512], f32, tag="pm")
                for k in range(8):
                    nc.tensor.matmul(
                        ps[:, :cw],
                        bx_sb[k][:, m * 128:(m + 1) * 128],
                        yt_sb[k][:, c0:c0 + cw],
                        start=(k == 0), stop=(k == 7),
                    )
                if m < 16:
                    nc.vector.tensor_copy(yp_sb[m][:, c0:c0 + cw], ps[:, :cw])
                else:
                    nc.scalar.activation(
                        gy_sb[:, c0:c0 + cw], ps[:, :cw], AF.Identity,
                        bias=l0b_sb[:, 0:1],
                    )

        emit_C_m(16)  # gy first (needed only in F, but frees nothing later)
        for m in range(4):
            emit_C_m(m)

        # local scores: psum_p[p][:, e*8+bt] = s(batch bt*128+row, expert e)
        psum_p = [ppp.tile([128, 128], f32, tag=f"pp{p}", name=f"psum_p{p}")
                  for p in range(2)]
        psum_g = ppp.tile([128, 16], f32, tag="pg", name="psum_g")

        # ---- phase F: global discriminator, both passes ----
        for p in range(2):
            off = p
            sgn = -1.0 if p == 0 else 1.0
            for ci, (c0, cw) in enumerate(CH_C):
                ps = ppm.tile([128, 512], f32, tag="pm")
                for k in range(8):
                    nc.tensor.matmul(
                        ps[:, :cw],
                        l0wh_sb[:, k * 128:(k + 1) * 128],
                        hm_sb[k][:, off + c0:off + c0 + cw],
                        start=(k == 0), stop=(k == 7),
                    )
                z0 = ptr2.tile([128, 512], bf, tag="z0")
                nc.vector.scalar_tensor_tensor(
                    z0[:, :cw], ps[:, :cw], 0.0, gy_sb[:, c0:c0 + cw],
                    op0=OP.add, op1=OP.add)
                h0 = ptr2.tile([128, 512], bf, tag="h0")
                nc.scalar.activation(h0[:, :cw], z0[:, :cw], AF.Relu)
                ps1 = ppm.tile([128, 512], f32, tag="pm")
                nc.tensor.matmul(
                    ps1[:, :cw], l1w_sb[:], h0[:, :cw], start=True, stop=True)
                h1g = ptr2.tile([128, 512], bf, tag="h1g")
                nc.scalar.activation(
                    h1g[:, :cw], ps1[:, :cw], AF.Relu, bias=l1b_sb[:, 0:1])
                for bti in range(4):
                    bt = ci * 4 + bti
                    nc.tensor.matmul(
                        psum_g[:, p * 8 + bt:p * 8 + bt + 1],
                        h1g[:, bti * 128:(bti + 1) * 128],
                        l2w_sb[:, 0:1],
                        start=True, stop=True,
                    )

        # ---- phase D+E interleaved per expert ----
        m3_sb = [None] * 8
        for e in range(NI):
            # emit C for expert e+4 lazily: keeps the yp pool small and
            # overlaps the remaining y_part matmuls with the expert phase
            if e + 4 < 16 and yp_sb[e + 4] is None:
                emit_C_m(e + 4)

            t = e // 2
            po = 64 * (e % 2)
            if m3_sb[t] is None:
                m3t = pm3.tile([128, BSP], bf, tag="m3", name=f"m3_{t}")
                nc.sync.dma_start(m3t[:], m3p[t, :, :])
                m3_sb[t] = m3t

            # D_e: mA_e = M3_e @ A_e + lb1_e  -> [128, 1025] bf16
            ma_t = pma.tile([128, BSP], bf, tag="ma")
            for (c0, cw) in CH_P:
                ps = ppm.tile([128, 512], f32, tag="pm")
                nc.tensor.matmul(
                    ps[:, :cw],
                    acat_sb[po:po + 64, e * 128:(e + 1) * 128],
                    m3_sb[t][po:po + 64, c0:c0 + cw],
                    start=True, stop=True,
                )
                nc.scalar.activation(
                    ma_t[:, c0:c0 + cw], ps[:, :cw], AF.Identity,
                    bias=lb1_sb[:, e:e + 1])

            # E_e: both passes
            for p in range(2):
                off = p  # joint reads cols 0..1023, marginal cols 1..1024
                z1 = ptr4.tile([128, BS], bf, tag="zh")
                nc.vector.tensor_add(
                    z1[:], ma_t[:, off:off + BS], yp_sb[e][:])
                h1 = ptr4.tile([128, BS], bf, tag="zh")
                nc.scalar.activation(h1[:], z1[:], AF.Relu)
                h2 = ptr2.tile([128, BS], bf, tag="h2")
                for ci, (c0, cw) in enumerate(CH_C):
                    ps2 = ppm.tile([128, 512], f32, tag="pm")
                    nc.tensor.matmul(
                        ps2[:, :cw],
                        w2s_sb[:, e * 128:(e + 1) * 128],
                        h1[:, c0:c0 + cw],
                        start=True, stop=True,
                    )
                    nc.vector.tensor_scalar(
                        h2[:, c0:c0 + cw], ps2[:, :cw],
                        lb2_sb[:, e:e + 1], 0.0, op0=OP.add, op1=OP.max)
                # L3 transposed: h2 b-tile stationary, w3 col moving;
                # score for (expert e, batch tile bt) -> psum_p col e*8+bt.
                # b3[e] seeded by a K=1 rank-1 matmul (ones x b3r8).
                nc.tensor.matmul(
                    psum_p[p][:, e * 8:(e + 1) * 8],
                    ones_sb[0:1, :],
                    b3r8_sb[0:1, e * 8:(e + 1) * 8],
                    start=True, stop=False, skip_group_check=True,
                )
                for bt in range(8):
                    nc.tensor.matmul(
                        psum_p[p][:, e * 8 + bt:e * 8 + bt + 1],
                        h2[:, bt * 128:(bt + 1) * 128],
                        w3s_sb[:, e:e + 1],
                        start=False, stop=True, skip_group_check=True,
                    )

        # ---- local softplus reduction: acc col p ----
        for p in range(2):
            sgn = -1.0 if p == 0 else 1.0
            exl = ptr1.tile([128, 128], f32, tag="exl", name=f"exl{p}")
            nc.scalar.activation(exl[:], psum_p[p][:], AF.Exp, scale=sgn)
            spl = ptr1.tile([128, 128], f32, tag="spl", name=f"spl{p}")
            nc.scalar.activation(
                spl[:], exl[:], AF.Ln, bias=1.0,
                accum_out=acc_sb[:, p:p + 1])

        # ---- global softplus reduction: acc col 2+p ----
        for p in range(2):
            sgn = -1.0 if p == 0 else 1.0
            exg = ptr1.tile([128, 16], f32, tag="exg", name=f"exg{p}")
            nc.scalar.activation(
                exg[:, :8], psum_g[:, p * 8:(p + 1) * 8], AF.Exp,
                scale=sgn, bias=l2b_sb[:, p:p + 1])
            spg = ptr1.tile([128, 16], f32, tag="spg", name=f"spg{p}")
            nc.scalar.activation(
                spg[:, :8], exg[:, :8], AF.Ln, bias=1.0,
                accum_out=acc_sb[:, 2 + p:3 + p])

        # ---- output ----
        nc.sync.dma_start(acc[:], acc_sb[:])

    nc.finalize()
    return nc


def _prep_shared(inputs):
    """Weight repack (identical for all cores), fp32 -> bf16."""
    f32 = np.float32
    gw0 = np.asarray(inputs["gw0"], f32)
    gw1 = np.asarray(inputs["gw1"], f32)
    l0w = np.asarray(inputs["l0w"], f32)
    l1w = np.asarray(inputs["l1w"], f32)
    l2w = np.asarray(inputs["l2w"], f32)
    lW1 = np.asarray(inputs["lW1"], f32)
    lW2 = np.asarray(inputs["lW2"], f32)
    lW3 = np.asarray(inputs["lW3"], f32)
    gb0 = np.asarray(inputs["gb0"], f32)
    gb1 = np.asarray(inputs["gb1"], f32)
    l0b = np.asarray(inputs["l0b"], f32)
    l1b = np.asarray(inputs["l1b"], f32)
    l2b = np.asarray(inputs["l2b"], f32)
    lb1 = np.asarray(inputs["lb1"], f32)
    lb2 = np.asarray(inputs["lb2"], f32)
    lb3 = np.asarray(inputs["lb3"], f32)

    def pk(a, kb):  # [K, N] -> [128, (K/128)*N] col-block k = rows k*128..
        K, N = a.shape
        return np.ascontiguousarray(
            a.reshape(K // 128, 128, N).transpose(1, 0, 2).reshape(128, -1))

    bcatx = np.concatenate(
        [lW1[:, DN:, :].transpose(1, 0, 2).reshape(D, NI * 128), l0w[:D]], axis=1)
    sh = {
        "gw0p": pk(gw0, 128).astype(BF),
        "gw1p": pk(gw1, 128).astype(BF),
        "bxp": pk(bcatx, 128).astype(BF),
        "acatp": np.ascontiguousarray(np.concatenate([
            lW1[:, :DN, :].transpose(1, 0, 2).reshape(DN, NI * 128)] * 2,
            axis=0)).astype(BF),
        "w2sp": np.ascontiguousarray(
            lW2.transpose(1, 0, 2).reshape(128, NI * 128)).astype(BF),
        "w3sp": np.ascontiguousarray(lW3[:, :, 0].T).astype(BF),
        "l0whp": pk(l0w[D:], 128).astype(BF),
        "l1wp": l1w.astype(BF),
        "l2wp": l2w.astype(BF),
        "gb0c": np.ascontiguousarray(gb0.reshape(8, 128).T),
        "gb1c": np.ascontiguousarray(gb1.reshape(8, 128).T),
        "lb1c": np.ascontiguousarray(lb1.T),
        "lb2c": np.ascontiguousarray(lb2.T),
        "onesr": np.ones((1, 128), BF),
        "b3r8": np.repeat(lb3[:, 0], 8)[None, :].astype(BF),
        "l0bc": np.ascontiguousarray(l0b[:, None]),
        "l1bc": np.ascontiguousarray(l1b[:, None]),
        "l2bc2": np.ascontiguousarray(
            np.stack([np.full(128, -l2b[0], f32),
                      np.full(128, l2b[0], f32)], axis=1)),
    }
    return sh


def _prep_core(inputs, c):
    f32 = np.float32
    y = np.asarray(inputs["y"], f32)
    M = np.asarray(inputs["M"], f32)
    r0 = c * BS
    rows = np.arange(r0, r0 + BSP) % B  # 1025 rows incl. overlap
    Ms = M[rows]  # [1025, 1024]
    ys = y[r0:r0 + BS]  # [1024, 1024]
    yt = np.ascontiguousarray(ys.T).astype(BF)  # [1024 feat, 1024]
    mt = np.ascontiguousarray(Ms.T).astype(BF)  # [1024 feat, 1025]
    # expert-major M: m3t[e, p, b] = Ms[b, p*16+e]; packed 2 experts/tile
    m3t = np.ascontiguousarray(
        Ms.reshape(BSP, DN, NI).transpose(2, 1, 0)).astype(BF)  # [16,64,1025]
    return {
        "yt3": np.ascontiguousarray(yt.reshape(8, 128, BS)),
        "mt3": np.ascontiguousarray(mt.reshape(8, 128, BSP)),
        "m3p": np.ascontiguousarray(m3t.reshape(8, 128, BSP)),
    }


def combine_partials(accs):
    """accs: list of 8 [128, 8] fp32 arrays -> scalar loss (float64 math)."""
    a = np.stack([np.asarray(x, np.float64) for x in accs])  # [8,128,8]
    sl_j = a[:, :, 0].sum()
    sl_m = a[:, :, 1].sum()
    sg_j = a[:, :, 2].sum()
    sg_m = a[:, :, 3].sum()
    local = BETA * (sl_m + sl_j) / (B * NI)
    glob = ALPHA * (sg_m + sg_j) / B
    return np.float32(local + glob)


def make_in_maps(inputs):
    sh = _prep_shared(inputs)
    return [dict(sh, **_prep_core(inputs, c)) for c in range(NC)]


def get_runner():
    global _RUNNER
    if _RUNNER is None:
        _RUNNER = _build_nc()
    return _RUNNER


def kernel(**inputs) -> np.ndarray:
    from concourse.bass_utils import run_bass_kernel_spmd

    nc = get_runner()
    in_maps = make_in_maps(inputs)
    res = run_bass_kernel_spmd(nc, in_maps, list(range(NC)))
    return combine_partials([r["acc"] for r in res.results])


# revision 13
# speedup vs baseline: 1.2047x; 1.0946x over previous
"""DeepInfoMax loss kernel for 8 Trainium2 NeuronCores.

Strategy (hardcoded for B=8192, d=1024, n=16):
  - Data-parallel over batch: core c gets rows [c*1024, (c+1)*1024), plus ONE
    overlap row ((c+1)*1024 % B) of M so the global roll (M_prime) is exact.
  - Activations are kept feature-major ([features, batch]) on-chip so weights
    are the stationary matmul operand.
  - Algebraic sharing: net(M) (global discriminator's M-branch) and the
    y-contribution of the local experts' first layer commute with the batch
    roll, so both are computed ONCE and re-sliced for the joint/marginal pass.
  - bf16 matmuls with fp32 PSUM accumulation; softplus = ln(1+exp(x)) on the
    scalar engine with fused accumulation into per-core partial sums.
  - Host combines 8x [128,8] partial-sum tiles into the final scalar.
"""

import numpy as np
import ml_dtypes

B = 8192
D = 1024
NI = 16
DN = D // NI  # 64
NC = 8
BS = B // NC  # 1024
BSP = BS + 1  # 1025 (overlap col for the exact roll)
ALPHA = 0.5
BETA = 1.0

# column chunks over the 1025-wide (producer) and 1024-wide (consumer) phases
CH_P = [(0, 342), (342, 342), (684, 341)]
CH_C = [(0, 512), (512, 512)]

BF = ml_dtypes.bfloat16

_RUNNER = None  # cached (nc, run) so repeated kernel() calls don't rebuild


def _build_nc():
    import concourse.bass as bass
    import concourse.tile as tile
    import concourse.mybir as mybir
    from concourse import bacc
    from contextlib import ExitStack

    bf = mybir.dt.bfloat16
    f32 = mybir.dt.float32
    AF = mybir.ActivationFunctionType
    OP = mybir.AluOpType

    nc = bacc.Bacc()

    # ---- DRAM I/O ----
    yt3 = nc.dram_tensor("yt3", [8, 128, BS], bf, kind="ExternalInput")
    mt3 = nc.dram_tensor("mt3", [8, 128, BSP], bf, kind="ExternalInput")
    m3p = nc.dram_tensor("m3p", [8, 128, BSP], bf, kind="ExternalInput")
    gw0p = nc.dram_tensor("gw0p", [128, 8 * D], bf, kind="ExternalInput")
    gw1p = nc.dram_tensor("gw1p", [128, 8 * D], bf, kind="ExternalInput")
    bxp = nc.dram_tensor("bxp", [128, 8 * 2176], bf, kind="ExternalInput")
    acatp = nc.dram_tensor("acatp", [128, 2048], bf, kind="ExternalInput")
    w2sp = nc.dram_tensor("w2sp", [128, 2048], bf, kind="ExternalInput")
    w3sp = nc.dram_tensor("w3sp", [128, NI], bf, kind="ExternalInput")
    l0whp = nc.dram_tensor("l0whp", [128, 1024], bf, kind="ExternalInput")
    l1wp = nc.dram_tensor("l1wp", [128, 128], bf, kind="ExternalInput")
    l2wp = nc.dram_tensor("l2wp", [128, 1], bf, kind="ExternalInput")
    idmp = nc.dram_tensor("idmp", [128, 128], bf, kind="ExternalInput")
    gb0c = nc.dram_tensor("gb0c", [128, 8], f32, kind="ExternalInput")
    gb1c = nc.dram_tensor("gb1c", [128, 8], f32, kind="ExternalInput")
    lb1c = nc.dram_tensor("lb1c", [128, NI], f32, kind="ExternalInput")
    lb2c = nc.dram_tensor("lb2c", [128, NI], f32, kind="ExternalInput")
    onesr = nc.dram_tensor("onesr", [1, 128], bf, kind="ExternalInput")
    b3r8 = nc.dram_tensor("b3r8", [1, 128], bf, kind="ExternalInput")
    l0bc = nc.dram_tensor("l0bc", [128, 1], f32, kind="ExternalInput")
    l1bc = nc.dram_tensor("l1bc", [128, 1], f32, kind="ExternalInput")
    l2bc2 = nc.dram_tensor("l2bc2", [128, 2], f32, kind="ExternalInput")
    acc = nc.dram_tensor("acc", [128, 8], f32, kind="ExternalOutput")

    with tile.TileContext(nc) as tc, ExitStack() as ctx:
        pconst = ctx.enter_context(tc.tile_pool(name="const", bufs=1))
        pgw = ctx.enter_context(tc.tile_pool(name="gw", bufs=16))
        pbx = ctx.enter_context(tc.tile_pool(name="bx", bufs=8))
        pac = ctx.enter_context(tc.tile_pool(name="ac", bufs=1))
        pyt = ctx.enter_context(tc.tile_pool(name="yt", bufs=8))
        p25 = ctx.enter_context(tc.tile_pool(name="t25", bufs=16))
        pm3 = ctx.enter_context(tc.tile_pool(name="m3", bufs=3))
        pyp = ctx.enter_context(tc.tile_pool(name="yp", bufs=6))
        pgy = ctx.enter_context(tc.tile_pool(name="gy", bufs=1))
        ptr4 = ctx.enter_context(tc.tile_pool(name="tr4", bufs=4))
        ptr2 = ctx.enter_context(tc.tile_pool(name="tr2", bufs=4))
        ptr1 = ctx.enter_context(tc.tile_pool(name="tr1", bufs=1))
        ppm = ctx.enter_context(tc.tile_pool(name="pm", bufs=5, space="PSUM"))
        ppp = ctx.enter_context(tc.tile_pool(name="pp", bufs=1, space="PSUM"))

        # ---- constants into SBUF ----
        def cload(dram, shape, dt):
            t = pconst.tile(shape, dt, tag=dram.name, name=dram.name + "_sb")
            nc.sync.dma_start(t[:], dram[:])
            return t

        w3s_sb = cload(w3sp, [128, NI], bf)
        l1w_sb = cload(l1wp, [128, 128], bf)
        l2w_sb = cload(l2wp, [128, 1], bf)
        idm_sb = cload(idmp, [128, 128], bf)
        l0wh_sb = cload(l0whp, [128, 1024], bf)
        w2s_sb = pac.tile([128, 2048], bf, tag="w2s")
        nc.sync.dma_start(w2s_sb[:], w2sp[:])
        gb0_sb = cload(gb0c, [128, 8], f32)
        gb1_sb = cload(gb1c, [128, 8], f32)
        lb1_sb = cload(lb1c, [128, NI], f32)
        lb2_sb = cload(lb2c, [128, NI], f32)
        ones_sb = cload(onesr, [1, 128], bf)
        b3r8_sb = cload(b3r8, [1, 128], bf)
        l0b_sb = cload(l0bc, [128, 1], f32)
        l1b_sb = cload(l1bc, [128, 1], f32)
        l2b_sb = cload(l2bc2, [128, 2], f32)
        acc_sb = pconst.tile([128, 8], f32, tag="acc")
        nc.vector.memset(acc_sb[:], 0.0)

        # ---- phase A inputs (k-granular DMAs so compute starts early) ----
        gw0_sb = []
        mt_sb = []
        for k in range(8):
            t = p25.tile([128, BSP], bf, tag="t25", name=f"mt_{k}")
            nc.sync.dma_start(t[:], mt3[k, :, :])
            mt_sb.append(t)
            g = pgw.tile([128, D], bf, tag="gw", name=f"gw0_{k}")
            nc.sync.dma_start(g[:], gw0p[:, k * D:(k + 1) * D])
            gw0_sb.append(g)

        # ---- phase A: h_g = relu(M @ gw0 + gb0), feature-major, 1025 cols ----
        hg_sb = []
        for m in range(8):
            t = p25.tile([128, BSP], bf, tag="t25", name=f"hg_{m}")
            hg_sb.append(t)
        for m in range(8):
            for (c0, cw) in CH_P:
                ps = ppm.tile([128, 512], f32, tag="pm")
                for k in range(8):
                    nc.tensor.matmul(
                        ps[:, :cw],
                        gw0_sb[k][:, m * 128:(m + 1) * 128],
                        mt_sb[k][:, c0:c0 + cw],
                        start=(k == 0), stop=(k == 7),
                    )
                nc.scalar.activation(
                    hg_sb[m][:, c0:c0 + cw], ps[:, :cw], AF.Relu,
                    bias=gb0_sb[:, m:m + 1],
                )

        # prefetch gw1 (k-granular)
        gw1_sb = []
        for k in range(8):
            g = pgw.tile([128, D], bf, tag="gw", name=f"gw1_{k}")
            nc.sync.dma_start(g[:], gw1p[:, k * D:(k + 1) * D])
            gw1_sb.append(g)

        # ---- phase B: hM = h_g @ gw1 + gb1 (no relu), 1025 cols ----
        hm_sb = []
        for m in range(8):
            t = p25.tile([128, BSP], bf, tag="t25", name=f"hm_{m}")
            hm_sb.append(t)
        for m in range(8):
            for (c0, cw) in CH_P:
                ps = ppm.tile([128, 512], f32, tag="pm")
                for k in range(8):
                    nc.tensor.matmul(
                        ps[:, :cw],
                        gw1_sb[k][:, m * 128:(m + 1) * 128],
                        hg_sb[k][:, c0:c0 + cw],
                        start=(k == 0), stop=(k == 7),
                    )
                nc.scalar.activation(
                    hm_sb[m][:, c0:c0 + cw], ps[:, :cw], AF.Identity,
                    bias=gb1_sb[:, m:m + 1],
                )

        # prefetch phase C inputs (k-granular)
        bx_sb = []
        for k in range(8):
            t = pbx.tile([128, 2176], bf, tag="bx", name=f"bx_{k}")
            nc.sync.dma_start(t[:], bxp[:, k * 2176:(k + 1) * 2176])
            bx_sb.append(t)
        yt_sb = []
        for k in range(8):
            t = pyt.tile([128, BS], bf, tag="yt", name=f"yt_{k}")
            nc.sync.dma_start(t[:], yt3[k, :, :])
            yt_sb.append(t)
        acat_sb = pac.tile([128, 2048], bf, tag="acat")
        nc.sync.dma_start(acat_sb[:], acatp[:])

        # ---- phase C: y_part (m 0..15) and gy (m 16), 1024 cols ----
        # yp[m] = (y @ Bcat)[:, m-block]^T ; gy = y @ l0w[:1024] + l0b (fp32)
        gy_sb = pgy.tile([128, BS], f32, tag="gy")
        yp_sb = [None] * 16

        def emit_C_m(m):
            if m < 16:
                yp_sb[m] = pyp.tile([128, BS], bf, tag="yp", name=f"yp_{m}")
            for (c0, cw) in CH_C:
                ps = ppm.tile([128, 512], f32, tag="pm")
                for k in range(8):
                    nc.tensor.matmul(
                        ps[:, :cw],
                        bx_sb[k][:, m * 128:(m + 1) * 128],
                        yt_sb[k][:, c0:c0 + cw],
                        start=(k == 0), stop=(k == 7),
                    )
                if m < 16:
                    nc.vector.tensor_copy(yp_sb[m][:, c0:c0 + cw], ps[:, :cw])
                else:
                    nc.scalar.activation(
                        gy_sb[:, c0:c0 + cw], ps[:, :cw], AF.Identity,
                        bias=l0b_sb[:, 0:1],
                    )

        emit_C_m(16)  # gy first (needed only in F, but frees nothing later)
        for m in range(4):
            emit_C_m(m)

        # local scores: psum_p[p][:, e*8+bt] = s(batch bt*128+row, expert e)
        psum_p = [ppp.tile([128, 128], f32, tag=f"pp{p}", name=f"psum_p{p}")
                  for p in range(2)]
        psum_g = ppp.tile([128, 16], f32, tag="pg", name="psum_g")

        # ---- phase F: global discriminator, both passes ----
        for p in range(2):
            off = p
            sgn = -1.0 if p == 0 else 1.0
            for ci, (c0, cw) in enumerate(CH_C):
                ps = ppm.tile([128, 512], f32, tag="pm")
                for k in range(8):
                    nc.tensor.matmul(
                        ps[:, :cw],
                        l0wh_sb[:, k * 128:(k + 1) * 128],
                        hm_sb[k][:, off + c0:off + c0 + cw],
                        start=(k == 0), stop=(k == 7),
                    )
                z0 = ptr2.tile([128, 512], bf, tag="z0")
                nc.vector.scalar_tensor_tensor(
                    z0[:, :cw], ps[:, :cw], 0.0, gy_sb[:, c0:c0 + cw],
                    op0=OP.add, op1=OP.add)
                h0 = ptr2.tile([128, 512], bf, tag="h0")
                nc.scalar.activation(h0[:, :cw], z0[:, :cw], AF.Relu)
                ps1 = ppm.tile([128, 512], f32, tag="pm")
                nc.tensor.matmul(
                    ps1[:, :cw], l1w_sb[:], h0[:, :cw], start=True, stop=True)
                h1g = ptr2.tile([128, 512], bf, tag="h1g")
                nc.scalar.activation(
                    h1g[:, :cw], ps1[:, :cw], AF.Relu, bias=l1b_sb[:, 0:1])
                for bti in range(4):
                    bt = ci * 4 + bti
                    nc.tensor.matmul(
                        psum_g[:, p * 8 + bt:p * 8 + bt + 1],
                        h1g[:, bti * 128:(bti + 1) * 128],
                        l2w_sb[:, 0:1],
                        start=True, stop=True,
                    )

        # ---- expert phase: z1 = yp + mA (+b1) built in PSUM ----
        # psum := I.T @ yp_chunk  (seed)  +  A_e.T @ M3_chunk ; ACT evicts
        # relu(psum + b1) -> h1; L2 on PE; DVE evicts relu(z2+b2) -> h2;
        # transposed L3 into psum_p columns.
        m3_sb = [None] * 8
        for e in range(NI):
            # emit C for expert e+4 lazily: keeps the yp pool small and
            # overlaps the remaining y_part matmuls with the expert phase
            if e + 4 < 16 and yp_sb[e + 4] is None:
                emit_C_m(e + 4)

            t = e // 2
            po = 64 * (e % 2)
            if m3_sb[t] is None:
                m3t = pm3.tile([128, BSP], bf, tag="m3", name=f"m3_{t}")
                nc.sync.dma_start(m3t[:], m3p[t, :, :])
                m3_sb[t] = m3t

            for p in range(2):
                off = p  # joint reads cols 0..1023, marginal cols 1..1024
                h1 = ptr4.tile([128, BS], bf, tag="h1", name=f"h1_{e}_{p}")
                for ci, (c0, cw) in enumerate(CH_C):
                    ps = ppm.tile([128, 512], f32, tag="pm")
                    nc.tensor.matmul(
                        ps[:, :cw], idm_sb[:],
                        yp_sb[e][:, c0:c0 + cw],
                        start=True, stop=False,
                    )
                    nc.tensor.matmul(
                        ps[:, :cw],
                        acat_sb[po:po + 64, e * 128:(e + 1) * 128],
                        m3_sb[t][po:po + 64, off + c0:off + c0 + cw],
                        start=False, stop=True,
                    )
                    nc.scalar.activation(
                        h1[:, c0:c0 + cw], ps[:, :cw], AF.Relu,
                        bias=lb1_sb[:, e:e + 1])
                h2 = ptr2.tile([128, BS], bf, tag="h2", name=f"h2_{e}_{p}")
                for ci, (c0, cw) in enumerate(CH_C):
                    ps2 = ppm.tile([128, 512], f32, tag="pm")
                    nc.tensor.matmul(
                        ps2[:, :cw],
                        w2s_sb[:, e * 128:(e + 1) * 128],
                        h1[:, c0:c0 + cw],
                        start=True, stop=True,
                    )
                    nc.vector.tensor_scalar(
                        h2[:, c0:c0 + cw], ps2[:, :cw],
                        lb2_sb[:, e:e + 1], 0.0, op0=OP.add, op1=OP.max)
                # L3 transposed: h2 b-tile stationary, w3 col moving;
                # score for (expert e, batch tile bt) -> psum_p col e*8+bt.
                # b3[e] seeded by a K=1 rank-1 matmul (ones x b3r8).
                nc.tensor.matmul(
                    psum_p[p][:, e * 8:(e + 1) * 8],
                    ones_sb[0:1, :],
                    b3r8_sb[0:1, e * 8:(e + 1) * 8],
                    start=True, stop=False, skip_group_check=True,
                )
                for bt in range(8):
                    nc.tensor.matmul(
                        psum_p[p][:, e * 8 + bt:e * 8 + bt + 1],
                        h2[:, bt * 128:(bt + 1) * 128],
                        w3s_sb[:, e:e + 1],
                        start=False, stop=True, skip_group_check=True,
                    )

        # ---- local softplus reduction: acc col p ----
        for p in range(2):
            sgn = -1.0 if p == 0 else 1.0
            exl = ptr1.tile([128, 128], f32, tag="exl", name=f"exl{p}")
            nc.scalar.activation(exl[:], psum_p[p][:], AF.Exp, scale=sgn)
            spl = ptr1.tile([128, 128], f32, tag="spl", name=f"spl{p}")
            nc.scalar.activation(
                spl[:], exl[:], AF.Ln, bias=1.0,
                accum_out=acc_sb[:, p:p + 1])

        # ---- global softplus reduction: acc col 2+p ----
        for p in range(2):
            sgn = -1.0 if p == 0 else 1.0
            exg = ptr1.tile([128, 16], f32, tag="exg", name=f"exg{p}")
            nc.scalar.activation(
                exg[:, :8], psum_g[:, p * 8:(p + 1) * 8], AF.Exp,
                scale=sgn, bias=l2b_sb[:, p:p + 1])
            spg = ptr1.tile([128, 16], f32, tag="spg", name=f"spg{p}")
            nc.scalar.activation(
                spg[:, :8], exg[:, :8], AF.Ln, bias=1.0,
                accum_out=acc_sb[:, 2 + p:3 + p])

        # ---- output ----
        nc.sync.dma_start(acc[:], acc_sb[:])

    nc.finalize()
    return nc


def _prep_shared(inputs):
    """Weight repack (identical for all cores), fp32 -> bf16."""
    f32 = np.float32
    gw0 = np.asarray(inputs["gw0"], f32)
    gw1 = np.asarray(inputs["gw1"], f32)
    l0w = np.asarray(inputs["l0w"], f32)
    l1w = np.asarray(inputs["l1w"], f32)
    l2w = np.asarray(inputs["l2w"], f32)
    lW1 = np.asarray(inputs["lW1"], f32)
    lW2 = np.asarray(inputs["lW2"], f32)
    lW3 = np.asarray(inputs["lW3"], f32)
    gb0 = np.asarray(inputs["gb0"], f32)
    gb1 = np.asarray(inputs["gb1"], f32)
    l0b = np.asarray(inputs["l0b"], f32)
    l1b = np.asarray(inputs["l1b"], f32)
    l2b = np.asarray(inputs["l2b"], f32)
    lb1 = np.asarray(inputs["lb1"], f32)
    lb2 = np.asarray(inputs["lb2"], f32)
    lb3 = np.asarray(inputs["lb3"], f32)

    def pk(a, kb):  # [K, N] -> [128, (K/128)*N] col-block k = rows k*128..
        K, N = a.shape
        return np.ascontiguousarray(
            a.reshape(K // 128, 128, N).transpose(1, 0, 2).reshape(128, -1))

    bcatx = np.concatenate(
        [lW1[:, DN:, :].transpose(1, 0, 2).reshape(D, NI * 128), l0w[:D]], axis=1)
    sh = {
        "gw0p": pk(gw0, 128).astype(BF),
        "gw1p": pk(gw1, 128).astype(BF),
        "bxp": pk(bcatx, 128).astype(BF),
        "acatp": np.ascontiguousarray(np.concatenate([
            lW1[:, :DN, :].transpose(1, 0, 2).reshape(DN, NI * 128)] * 2,
            axis=0)).astype(BF),
        "w2sp": np.ascontiguousarray(
            lW2.transpose(1, 0, 2).reshape(128, NI * 128)).astype(BF),
        "w3sp": np.ascontiguousarray(lW3[:, :, 0].T).astype(BF),
        "l0whp": pk(l0w[D:], 128).astype(BF),
        "l1wp": l1w.astype(BF),
        "l2wp": l2w.astype(BF),
        "idmp": np.eye(128, dtype=np.float32).astype(BF),
        "gb0c": np.ascontiguousarray(gb0.reshape(8, 128).T),
        "gb1c": np.ascontiguousarray(gb1.reshape(8, 128).T),
        "lb1c": np.ascontiguousarray(lb1.T),
        "lb2c": np.ascontiguousarray(lb2.T),
        "onesr": np.ones((1, 128), BF),
        "b3r8": np.repeat(lb3[:, 0], 8)[None, :].astype(BF),
        "l0bc": np.ascontiguousarray(l0b[:, None]),
        "l1bc": np.ascontiguousarray(l1b[:, None]),
        "l2bc2": np.ascontiguousarray(
            np.stack([np.full(128, -l2b[0], f32),
                      np.full(128, l2b[0], f32)], axis=1)),
    }
    return sh


def _prep_core(inputs, c):
    f32 = np.float32
    y = np.asarray(inputs["y"], f32)
    M = np.asarray(inputs["M"], f32)
    r0 = c * BS
    rows = np.arange(r0, r0 + BSP) % B  # 1025 rows incl. overlap
    Ms = M[rows]  # [1025, 1024]
    ys = y[r0:r0 + BS]  # [1024, 1024]
    yt = np.ascontiguousarray(ys.T).astype(BF)  # [1024 feat, 1024]
    mt = np.ascontiguousarray(Ms.T).astype(BF)  # [1024 feat, 1025]
    # expert-major M: m3t[e, p, b] = Ms[b, p*16+e]; packed 2 experts/tile
    m3t = np.ascontiguousarray(
        Ms.reshape(BSP, DN, NI).transpose(2, 1, 0)).astype(BF)  # [16,64,1025]
    return {
        "yt3": np.ascontiguousarray(yt.reshape(8, 128, BS)),
        "mt3": np.ascontiguousarray(mt.reshape(8, 128, BSP)),
        "m3p": np.ascontiguousarray(m3t.reshape(8, 128, BSP)),
    }


def combine_partials(accs):
    """accs: list of 8 [128, 8] fp32 arrays -> scalar loss (float64 math)."""
    a = np.stack([np.asarray(x, np.float64) for x in accs])  # [8,128,8]
    sl_j = a[:, :, 0].sum()
    sl_m = a[:, :, 1].sum()
    sg_j = a[:, :, 2].sum()
    sg_m = a[:, :, 3].sum()
    local = BETA * (sl_m + sl_j) / (B * NI)
    glob = ALPHA * (sg_m + sg_j) / B
    return np.float32(local + glob)


def make_in_maps(inputs):
    sh = _prep_shared(inputs)
    return [dict(sh, **_prep_core(inputs, c)) for c in range(NC)]


def get_runner():
    global _RUNNER
    if _RUNNER is None:
        _RUNNER = _build_nc()
    return _RUNNER


def kernel(**inputs) -> np.ndarray:
    from concourse.bass_utils import run_bass_kernel_spmd

    nc = get_runner()
    in_maps = make_in_maps(inputs)
    res = run_bass_kernel_spmd(nc, in_maps, list(range(NC)))
    return combine_partials([r["acc"] for r in res.results])


# revision 14
# speedup vs baseline: 1.2188x; 1.0117x over previous
"""DeepInfoMax loss kernel for 8 Trainium2 NeuronCores.

Strategy (hardcoded for B=8192, d=1024, n=16):
  - Data-parallel over batch: core c gets rows [c*1024, (c+1)*1024), plus ONE
    overlap row ((c+1)*1024 % B) of M so the global roll (M_prime) is exact.
  - Activations are kept feature-major ([features, batch]) on-chip so weights
    are the stationary matmul operand.
  - Algebraic sharing: net(M) (global discriminator's M-branch) and the
    y-contribution of the local experts' first layer commute with the batch
    roll, so both are computed ONCE and re-sliced for the joint/marginal pass.
  - bf16 matmuls with fp32 PSUM accumulation; softplus = ln(1+exp(x)) on the
    scalar engine with fused accumulation into per-core partial sums.
  - Host combines 8x [128,8] partial-sum tiles into the final scalar.
"""

import numpy as np
import ml_dtypes

B = 8192
D = 1024
NI = 16
DN = D // NI  # 64
NC = 8
BS = B // NC  # 1024
BSP = BS + 1  # 1025 (overlap col for the exact roll)
ALPHA = 0.5
BETA = 1.0

# column chunks over the 1025-wide (producer) and 1024-wide (consumer) phases
CH_P = [(0, 342), (342, 342), (684, 341)]
CH_C = [(0, 512), (512, 512)]

BF = ml_dtypes.bfloat16

_RUNNER = None  # cached (nc, run) so repeated kernel() calls don't rebuild


def _build_nc():
    import concourse.bass as bass
    import concourse.tile as tile
    import concourse.mybir as mybir
    from concourse import bacc
    from contextlib import ExitStack

    bf = mybir.dt.bfloat16
    f32 = mybir.dt.float32
    AF = mybir.ActivationFunctionType
    OP = mybir.AluOpType

    nc = bacc.Bacc()

    # ---- DRAM I/O ----
    yt3 = nc.dram_tensor("yt3", [8, 128, BS], bf, kind="ExternalInput")
    mt3 = nc.dram_tensor("mt3", [8, 128, BSP], bf, kind="ExternalInput")
    m3p = nc.dram_tensor("m3p", [8, 128, BSP], bf, kind="ExternalInput")
    gw0p = nc.dram_tensor("gw0p", [128, 8 * D], bf, kind="ExternalInput")
    gw1p = nc.dram_tensor("gw1p", [128, 8 * D], bf, kind="ExternalInput")
    bxp = nc.dram_tensor("bxp", [128, 8 * 2176], bf, kind="ExternalInput")
    acatp = nc.dram_tensor("acatp", [128, 2048], bf, kind="ExternalInput")
    w2sp = nc.dram_tensor("w2sp", [128, 2048], bf, kind="ExternalInput")
    w3sp = nc.dram_tensor("w3sp", [128, NI], bf, kind="ExternalInput")
    l0whp = nc.dram_tensor("l0whp", [128, 1024], bf, kind="ExternalInput")
    l1wp = nc.dram_tensor("l1wp", [128, 128], bf, kind="ExternalInput")
    l2wp = nc.dram_tensor("l2wp", [128, 1], bf, kind="ExternalInput")
    idmp = nc.dram_tensor("idmp", [128, 128], bf, kind="ExternalInput")
    gb0c = nc.dram_tensor("gb0c", [128, 8], f32, kind="ExternalInput")
    gb1c = nc.dram_tensor("gb1c", [128, 8], f32, kind="ExternalInput")
    lb1c = nc.dram_tensor("lb1c", [128, NI], f32, kind="ExternalInput")
    lb2c = nc.dram_tensor("lb2c", [128, NI], f32, kind="ExternalInput")
    onesr = nc.dram_tensor("onesr", [1, 128], bf, kind="ExternalInput")
    b3r8 = nc.dram_tensor("b3r8", [1, 128], bf, kind="ExternalInput")
    l0bc = nc.dram_tensor("l0bc", [128, 1], f32, kind="ExternalInput")
    l1bc = nc.dram_tensor("l1bc", [128, 1], f32, kind="ExternalInput")
    l2bc2 = nc.dram_tensor("l2bc2", [128, 2], f32, kind="ExternalInput")
    acc = nc.dram_tensor("acc", [128, 8], f32, kind="ExternalOutput")

    with tile.TileContext(nc) as tc, ExitStack() as ctx:
        pconst = ctx.enter_context(tc.tile_pool(name="const", bufs=1))
        pgw = ctx.enter_context(tc.tile_pool(name="gw", bufs=16))
        pbx = ctx.enter_context(tc.tile_pool(name="bx", bufs=8))
        pac = ctx.enter_context(tc.tile_pool(name="ac", bufs=1))
        pyt = ctx.enter_context(tc.tile_pool(name="yt", bufs=8))
        p25 = ctx.enter_context(tc.tile_pool(name="t25", bufs=16))
        pm3 = ctx.enter_context(tc.tile_pool(name="m3", bufs=4))
        pyp = ctx.enter_context(tc.tile_pool(name="yp", bufs=8))
        pgy = ctx.enter_context(tc.tile_pool(name="gy", bufs=1))
        ptr4 = ctx.enter_context(tc.tile_pool(name="tr4", bufs=4))
        ptr2 = ctx.enter_context(tc.tile_pool(name="tr2", bufs=4))
        ptr1 = ctx.enter_context(tc.tile_pool(name="tr1", bufs=1))
        ppm = ctx.enter_context(tc.tile_pool(name="pm", bufs=5, space="PSUM"))
        ppp = ctx.enter_context(tc.tile_pool(name="pp", bufs=1, space="PSUM"))

        # ---- phase A inputs (k-granular DMAs so compute starts early) ----
        gw0_sb = []
        mt_sb = []
        for k in range(8):
            t = p25.tile([128, BSP], bf, tag="t25", name=f"mt_{k}")
            nc.sync.dma_start(t[:], mt3[k, :, :])
            mt_sb.append(t)
            g = pgw.tile([128, D], bf, tag="gw", name=f"gw0_{k}")
            nc.sync.dma_start(g[:], gw0p[:, k * D:(k + 1) * D])
            gw0_sb.append(g)

        # ---- constants into SBUF ----
        def cload(dram, shape, dt):
            t = pconst.tile(shape, dt, tag=dram.name, name=dram.name + "_sb")
            nc.gpsimd.dma_start(t[:], dram[:])
            return t

        w3s_sb = cload(w3sp, [128, NI], bf)
        l1w_sb = cload(l1wp, [128, 128], bf)
        l2w_sb = cload(l2wp, [128, 1], bf)
        idm_sb = cload(idmp, [128, 128], bf)
        l0wh_sb = cload(l0whp, [128, 1024], bf)
        w2s_sb = pac.tile([128, 2048], bf, tag="w2s")
        nc.gpsimd.dma_start(w2s_sb[:], w2sp[:])
        gb0_sb = cload(gb0c, [128, 8], f32)
        gb1_sb = cload(gb1c, [128, 8], f32)
        lb1_sb = cload(lb1c, [128, NI], f32)
        lb2_sb = cload(lb2c, [128, NI], f32)
        ones_sb = cload(onesr, [1, 128], bf)
        b3r8_sb = cload(b3r8, [1, 128], bf)
        l0b_sb = cload(l0bc, [128, 1], f32)
        l1b_sb = cload(l1bc, [128, 1], f32)
        l2b_sb = cload(l2bc2, [128, 2], f32)
        acc_sb = pconst.tile([128, 8], f32, tag="acc")
        nc.vector.memset(acc_sb[:], 0.0)

        # ---- phase A: h_g = relu(M @ gw0 + gb0), feature-major, 1025 cols ----
        hg_sb = []
        for m in range(8):
            t = p25.tile([128, BSP], bf, tag="t25", name=f"hg_{m}")
            hg_sb.append(t)
        for m in range(8):
            for (c0, cw) in CH_P:
                ps = ppm.tile([128, 512], f32, tag="pm")
                for k in range(8):
                    nc.tensor.matmul(
                        ps[:, :cw],
                        gw0_sb[k][:, m * 128:(m + 1) * 128],
                        mt_sb[k][:, c0:c0 + cw],
                        start=(k == 0), stop=(k == 7),
                    )
                nc.scalar.activation(
                    hg_sb[m][:, c0:c0 + cw], ps[:, :cw], AF.Relu,
                    bias=gb0_sb[:, m:m + 1],
                )

        # prefetch gw1 (k-granular)
        gw1_sb = []
        for k in range(8):
            g = pgw.tile([128, D], bf, tag="gw", name=f"gw1_{k}")
            nc.sync.dma_start(g[:], gw1p[:, k * D:(k + 1) * D])
            gw1_sb.append(g)

        # ---- phase B: hM = h_g @ gw1 + gb1 (no relu), 1025 cols ----
        hm_sb = []
        for m in range(8):
            t = p25.tile([128, BSP], bf, tag="t25", name=f"hm_{m}")
            hm_sb.append(t)
        for m in range(8):
            for (c0, cw) in CH_P:
                ps = ppm.tile([128, 512], f32, tag="pm")
                for k in range(8):
                    nc.tensor.matmul(
                        ps[:, :cw],
                        gw1_sb[k][:, m * 128:(m + 1) * 128],
                        hg_sb[k][:, c0:c0 + cw],
                        start=(k == 0), stop=(k == 7),
                    )
                nc.scalar.activation(
                    hm_sb[m][:, c0:c0 + cw], ps[:, :cw], AF.Identity,
                    bias=gb1_sb[:, m:m + 1],
                )

        # prefetch phase C inputs (k-granular)
        bx_sb = []
        for k in range(8):
            t = pbx.tile([128, 2176], bf, tag="bx", name=f"bx_{k}")
            nc.sync.dma_start(t[:], bxp[:, k * 2176:(k + 1) * 2176])
            bx_sb.append(t)
        yt_sb = []
        for k in range(8):
            t = pyt.tile([128, BS], bf, tag="yt", name=f"yt_{k}")
            nc.sync.dma_start(t[:], yt3[k, :, :])
            yt_sb.append(t)
        acat_sb = pac.tile([128, 2048], bf, tag="acat")
        nc.gpsimd.dma_start(acat_sb[:], acatp[:])

        # ---- phase C: y_part (m 0..15) and gy (m 16), 1024 cols ----
        # yp[m] = (y @ Bcat)[:, m-block]^T ; gy = y @ l0w[:1024] + l0b (fp32)
        gy_sb = pgy.tile([128, BS], f32, tag="gy")
        yp_sb = [None] * 16

        def emit_C_m(m):
            if m < 16:
                yp_sb[m] = pyp.tile([128, BS], bf, tag="yp", name=f"yp_{m}")
            for (c0, cw) in CH_C:
                ps = ppm.tile([128, 512], f32, tag="pm")
                for k in range(8):
                    nc.tensor.matmul(
                        ps[:, :cw],
                        bx_sb[k][:, m * 128:(m + 1) * 128],
                        yt_sb[k][:, c0:c0 + cw],
                        start=(k == 0), stop=(k == 7),
                    )
                if m < 16:
                    nc.vector.tensor_copy(yp_sb[m][:, c0:c0 + cw], ps[:, :cw])
                else:
                    nc.scalar.activation(
                        gy_sb[:, c0:c0 + cw], ps[:, :cw], AF.Identity,
                        bias=l0b_sb[:, 0:1],
                    )

        emit_C_m(16)  # gy first (needed only in F, but frees nothing later)
        for m in range(4):
            emit_C_m(m)

        # local scores: psum_p[p][:, e*8+bt] = s(batch bt*128+row, expert e)
        psum_p = [ppp.tile([128, 128], f32, tag=f"pp{p}", name=f"psum_p{p}")
                  for p in range(2)]
        psum_g = ppp.tile([128, 16], f32, tag="pg", name="psum_g")

        # ---- phase F: global discriminator, both passes ----
        for p in range(2):
            off = p
            sgn = -1.0 if p == 0 else 1.0
            for ci, (c0, cw) in enumerate(CH_C):
                ps = ppm.tile([128, 512], f32, tag="pm")
                for k in range(8):
                    nc.tensor.matmul(
                        ps[:, :cw],
                        l0wh_sb[:, k * 128:(k + 1) * 128],
                        hm_sb[k][:, off + c0:off + c0 + cw],
                        start=(k == 0), stop=(k == 7),
                    )
                z0 = ptr2.tile([128, 512], bf, tag="z0")
                nc.vector.scalar_tensor_tensor(
                    z0[:, :cw], ps[:, :cw], 0.0, gy_sb[:, c0:c0 + cw],
                    op0=OP.add, op1=OP.add)
                h0 = ptr2.tile([128, 512], bf, tag="h0")
                nc.scalar.activation(h0[:, :cw], z0[:, :cw], AF.Relu)
                ps1 = ppm.tile([128, 512], f32, tag="pm")
                nc.tensor.matmul(
                    ps1[:, :cw], l1w_sb[:], h0[:, :cw], start=True, stop=True)
                h1g = ptr2.tile([128, 512], bf, tag="h1g")
                nc.scalar.activation(
                    h1g[:, :cw], ps1[:, :cw], AF.Relu, bias=l1b_sb[:, 0:1])
                for bti in range(4):
                    bt = ci * 4 + bti
                    nc.tensor.matmul(
                        psum_g[:, p * 8 + bt:p * 8 + bt + 1],
                        h1g[:, bti * 128:(bti + 1) * 128],
                        l2w_sb[:, 0:1],
                        start=True, stop=True,
                    )

        # ---- expert phase: z1 = yp + mA (+b1) built in PSUM ----
        # psum := I.T @ yp_chunk  (seed)  +  A_e.T @ M3_chunk ; ACT evicts
        # relu(psum + b1) -> h1; L2 on PE; DVE evicts relu(z2+b2) -> h2;
        # transposed L3 into psum_p columns.
        m3_sb = [None] * 8
        for t in range(2):
            m3t = pm3.tile([128, BSP], bf, tag="m3", name=f"m3_{t}")
            nc.sync.dma_start(m3t[:], m3p[t, :, :])
            m3_sb[t] = m3t
        for e in range(NI):
            # emit C for expert e+4 lazily: keeps the yp pool small and
            # overlaps the remaining y_part matmuls with the expert phase
            if e + 4 < 16 and yp_sb[e + 4] is None:
                emit_C_m(e + 4)

            t = e // 2
            po = 64 * (e % 2)
            if m3_sb[t] is None:
                m3t = pm3.tile([128, BSP], bf, tag="m3", name=f"m3_{t}")
                nc.sync.dma_start(m3t[:], m3p[t, :, :])
                m3_sb[t] = m3t

            for p in range(2):
                off = p  # joint reads cols 0..1023, marginal cols 1..1024
                h1 = ptr4.tile([128, BS], bf, tag="h1", name=f"h1_{e}_{p}")
                for ci, (c0, cw) in enumerate(CH_C):
                    ps = ppm.tile([128, 512], f32, tag="pm")
                    nc.tensor.matmul(
                        ps[:, :cw], idm_sb[:],
                        yp_sb[e][:, c0:c0 + cw],
                        start=True, stop=False,
                    )
                    nc.tensor.matmul(
                        ps[:, :cw],
                        acat_sb[po:po + 64, e * 128:(e + 1) * 128],
                        m3_sb[t][po:po + 64, off + c0:off + c0 + cw],
                        start=False, stop=True,
                    )
                    nc.scalar.activation(
                        h1[:, c0:c0 + cw], ps[:, :cw], AF.Relu,
                        bias=lb1_sb[:, e:e + 1])
                h2 = ptr2.tile([128, BS], bf, tag="h2", name=f"h2_{e}_{p}")
                for ci, (c0, cw) in enumerate(CH_C):
                    ps2 = ppm.tile([128, 512], f32, tag="pm")
                    nc.tensor.matmul(
                        ps2[:, :cw],
                        w2s_sb[:, e * 128:(e + 1) * 128],
                        h1[:, c0:c0 + cw],
                        start=True, stop=True,
                    )
                    nc.vector.tensor_scalar(
                        h2[:, c0:c0 + cw], ps2[:, :cw],
                        lb2_sb[:, e:e + 1], 0.0, op0=OP.add, op1=OP.max)
                # L3 transposed: h2 b-tile stationary, w3 col moving;
                # score for (expert e, batch tile bt) -> psum_p col e*8+bt.
                # b3[e] seeded by a K=1 rank-1 matmul (ones x b3r8).
                nc.tensor.matmul(
                    psum_p[p][:, e * 8:(e + 1) * 8],
                    ones_sb[0:1, :],
                    b3r8_sb[0:1, e * 8:(e + 1) * 8],
                    start=True, stop=False, skip_group_check=True,
                )
                for bt in range(8):
                    nc.tensor.matmul(
                        psum_p[p][:, e * 8 + bt:e * 8 + bt + 1],
                        h2[:, bt * 128:(bt + 1) * 128],
                        w3s_sb[:, e:e + 1],
                        start=False, stop=True, skip_group_check=True,
                    )

        # ---- local softplus reduction: acc col p ----
        for p in range(2):
            sgn = -1.0 if p == 0 else 1.0
            exl = ptr1.tile([128, 128], f32, tag="exl", name=f"exl{p}")
            nc.scalar.activation(exl[:], psum_p[p][:], AF.Exp, scale=sgn)
            spl = ptr1.tile([128, 128], f32, tag="spl", name=f"spl{p}")
            nc.scalar.activation(
                spl[:], exl[:], AF.Ln, bias=1.0,
                accum_out=acc_sb[:, p:p + 1])

        # ---- global softplus reduction: acc col 2+p ----
        for p in range(2):
            sgn = -1.0 if p == 0 else 1.0
            exg = ptr1.tile([128, 16], f32, tag="exg", name=f"exg{p}")
            nc.scalar.activation(
                exg[:, :8], psum_g[:, p * 8:(p + 1) * 8], AF.Exp,
                scale=sgn, bias=l2b_sb[:, p:p + 1])
            spg = ptr1.tile([128, 16], f32, tag="spg", name=f"spg{p}")
            nc.scalar.activation(
                spg[:, :8], exg[:, :8], AF.Ln, bias=1.0,
                accum_out=acc_sb[:, 2 + p:3 + p])

        # ---- output ----
        nc.sync.dma_start(acc[:], acc_sb[:])

    nc.finalize()
    return nc


def _prep_shared(inputs):
    """Weight repack (identical for all cores), fp32 -> bf16."""
    f32 = np.float32
    gw0 = np.asarray(inputs["gw0"], f32)
    gw1 = np.asarray(inputs["gw1"], f32)
    l0w = np.asarray(inputs["l0w"], f32)
    l1w = np.asarray(inputs["l1w"], f32)
    l2w = np.asarray(inputs["l2w"], f32)
    lW1 = np.asarray(inputs["lW1"], f32)
    lW2 = np.asarray(inputs["lW2"], f32)
    lW3 = np.asarray(inputs["lW3"], f32)
    gb0 = np.asarray(inputs["gb0"], f32)
    gb1 = np.asarray(inputs["gb1"], f32)
    l0b = np.asarray(inputs["l0b"], f32)
    l1b = np.asarray(inputs["l1b"], f32)
    l2b = np.asarray(inputs["l2b"], f32)
    lb1 = np.asarray(inputs["lb1"], f32)
    lb2 = np.asarray(inputs["lb2"], f32)
    lb3 = np.asarray(inputs["lb3"], f32)

    def pk(a, kb):  # [K, N] -> [128, (K/128)*N] col-block k = rows k*128..
        K, N = a.shape
        return np.ascontiguousarray(
            a.reshape(K // 128, 128, N).transpose(1, 0, 2).reshape(128, -1))

    bcatx = np.concatenate(
        [lW1[:, DN:, :].transpose(1, 0, 2).reshape(D, NI * 128), l0w[:D]], axis=1)
    sh = {
        "gw0p": pk(gw0, 128).astype(BF),
        "gw1p": pk(gw1, 128).astype(BF),
        "bxp": pk(bcatx, 128).astype(BF),
        "acatp": np.ascontiguousarray(np.concatenate([
            lW1[:, :DN, :].transpose(1, 0, 2).reshape(DN, NI * 128)] * 2,
            axis=0)).astype(BF),
        "w2sp": np.ascontiguousarray(
            lW2.transpose(1, 0, 2).reshape(128, NI * 128)).astype(BF),
        "w3sp": np.ascontiguousarray(lW3[:, :, 0].T).astype(BF),
        "l0whp": pk(l0w[D:], 128).astype(BF),
        "l1wp": l1w.astype(BF),
        "l2wp": l2w.astype(BF),
        "idmp": np.eye(128, dtype=np.float32).astype(BF),
        "gb0c": np.ascontiguousarray(gb0.reshape(8, 128).T),
        "gb1c": np.ascontiguousarray(gb1.reshape(8, 128).T),
        "lb1c": np.ascontiguousarray(lb1.T),
        "lb2c": np.ascontiguousarray(lb2.T),
        "onesr": np.ones((1, 128), BF),
        "b3r8": np.repeat(lb3[:, 0], 8)[None, :].astype(BF),
        "l0bc": np.ascontiguousarray(l0b[:, None]),
        "l1bc": np.ascontiguousarray(l1b[:, None]),
        "l2bc2": np.ascontiguousarray(
            np.stack([np.full(128, -l2b[0], f32),
                      np.full(128, l2b[0], f32)], axis=1)),
    }
    return sh


def _prep_core(inputs, c):
    f32 = np.float32
    y = np.asarray(inputs["y"], f32)
    M = np.asarray(inputs["M"], f32)
    r0 = c * BS
    rows = np.arange(r0, r0 + BSP) % B  # 1025 rows incl. overlap
    Ms = M[rows]  # [1025, 1024]
    ys = y[r0:r0 + BS]  # [1024, 1024]
    yt = np.ascontiguousarray(ys.T).astype(BF)  # [1024 feat, 1024]
    mt = np.ascontiguousarray(Ms.T).astype(BF)  # [1024 feat, 1025]
    # expert-major M: m3t[e, p, b] = Ms[b, p*16+e]; packed 2 experts/tile
    m3t = np.ascontiguousarray(
        Ms.reshape(BSP, DN, NI).transpose(2, 1, 0)).astype(BF)  # [16,64,1025]
    return {
        "yt3": np.ascontiguousarray(yt.reshape(8, 128, BS)),
        "mt3": np.ascontiguousarray(mt.reshape(8, 128, BSP)),
        "m3p": np.ascontiguousarray(m3t.reshape(8, 128, BSP)),
    }


def combine_partials(accs):
    """accs: list of 8 [128, 8] fp32 arrays -> scalar loss (float64 math)."""
    a = np.stack([np.asarray(x, np.float64) for x in accs])  # [8,128,8]
    sl_j = a[:, :, 0].sum()
    sl_m = a[:, :, 1].sum()
    sg_j = a[:, :, 2].sum()
    sg_m = a[:, :, 3].sum()
    local = BETA * (sl_m + sl_j) / (B * NI)
    glob = ALPHA * (sg_m + sg_j) / B
    return np.float32(local + glob)


def make_in_maps(inputs):
    sh = _prep_shared(inputs)
    return [dict(sh, **_prep_core(inputs, c)) for c in range(NC)]


def get_runner():
    global _RUNNER
    if _RUNNER is None:
        _RUNNER = _build_nc()
    return _RUNNER


def kernel(**inputs) -> np.ndarray:
    from concourse.bass_utils import run_bass_kernel_spmd

    nc = get_runner()
    in_maps = make_in_maps(inputs)
    res = run_bass_kernel_spmd(nc, in_maps, list(range(NC)))
    return combine_partials([r["acc"] for r in res.results])


# revision 15
# speedup vs baseline: 1.5926x; 1.3067x over previous
"""DeepInfoMax loss kernel for 8 Trainium2 NeuronCores.

Strategy (hardcoded for B=8192, d=1024, n=16):
  - Data-parallel over batch: core c gets rows [c*1024, (c+1)*1024), plus ONE
    overlap row ((c+1)*1024 % B) of M so the global roll (M_prime) is exact.
  - Activations are kept feature-major ([features, batch]) on-chip so weights
    are the stationary matmul operand.
  - Algebraic sharing: net(M) (global discriminator's M-branch) and the
    y-contribution of the local experts' first layer commute with the batch
    roll, so both are computed ONCE and re-sliced for the joint/marginal pass.
  - bf16 matmuls with fp32 PSUM accumulation; softplus = ln(1+exp(x)) on the
    scalar engine with fused accumulation into per-core partial sums.
  - Host combines 8x [128,8] partial-sum tiles into the final scalar.
"""

import numpy as np
import ml_dtypes

B = 8192
D = 1024
NI = 16
DN = D // NI  # 64
NC = 8
BS = B // NC  # 1024
BSP = BS + 1  # 1025 (overlap col for the exact roll)
ALPHA = 0.5
BETA = 1.0

# column chunks over the 1025-wide (producer) and 1024-wide (consumer) phases
CH_P = [(0, 342), (342, 342), (684, 341)]
CH_C = [(0, 512), (512, 512)]

BF = ml_dtypes.bfloat16
F8 = ml_dtypes.float8_e4m3
WSC = 64.0

_RUNNER = None  # cached (nc, run) so repeated kernel() calls don't rebuild


def _build_nc():
    import concourse.bass as bass
    import concourse.tile as tile
    import concourse.mybir as mybir
    from concourse import bacc
    from contextlib import ExitStack

    bf = mybir.dt.bfloat16
    f32 = mybir.dt.float32
    AF = mybir.ActivationFunctionType
    OP = mybir.AluOpType

    nc = bacc.Bacc()

    # ---- DRAM I/O ----
    f8 = mybir.dt.float8e4
    ytd = nc.dram_tensor("ytd", [4, 128, 2 * 1040], f8, kind="ExternalInput")
    mtd = nc.dram_tensor("mtd", [4, 128, 2 * 1040], f8, kind="ExternalInput")
    m3p = nc.dram_tensor("m3p", [8, 128, BSP], bf, kind="ExternalInput")
    gw0d = nc.dram_tensor("gw0d", [4, 128, 2 * D], f8, kind="ExternalInput")
    gw1d = nc.dram_tensor("gw1d", [4, 128, 2 * D], f8, kind="ExternalInput")
    bxd = nc.dram_tensor("bxd", [4, 128, 2 * 2176], f8, kind="ExternalInput")
    acatp = nc.dram_tensor("acatp", [128, 2048], bf, kind="ExternalInput")
    w2sp = nc.dram_tensor("w2sp", [128, 2048], bf, kind="ExternalInput")
    w3sp = nc.dram_tensor("w3sp", [128, NI], bf, kind="ExternalInput")
    l0whp = nc.dram_tensor("l0whp", [128, 1024], bf, kind="ExternalInput")
    l1wp = nc.dram_tensor("l1wp", [128, 128], bf, kind="ExternalInput")
    l2wp = nc.dram_tensor("l2wp", [128, 1], bf, kind="ExternalInput")
    idmp = nc.dram_tensor("idmp", [128, 128], bf, kind="ExternalInput")
    gb0c = nc.dram_tensor("gb0c", [128, 8], f32, kind="ExternalInput")
    gb1c = nc.dram_tensor("gb1c", [128, 8], f32, kind="ExternalInput")
    lb1c = nc.dram_tensor("lb1c", [128, NI], f32, kind="ExternalInput")
    lb2c = nc.dram_tensor("lb2c", [128, NI], f32, kind="ExternalInput")
    onesr = nc.dram_tensor("onesr", [1, 128], bf, kind="ExternalInput")
    b3r8 = nc.dram_tensor("b3r8", [1, 128], bf, kind="ExternalInput")
    l0bc = nc.dram_tensor("l0bc", [128, 1], f32, kind="ExternalInput")
    l1bc = nc.dram_tensor("l1bc", [128, 1], f32, kind="ExternalInput")
    l2bc2 = nc.dram_tensor("l2bc2", [128, 2], f32, kind="ExternalInput")
    acc = nc.dram_tensor("acc", [128, 8], f32, kind="ExternalOutput")

    with tile.TileContext(nc) as tc, ExitStack() as ctx:
        pconst = ctx.enter_context(tc.tile_pool(name="const", bufs=1))
        pgw = ctx.enter_context(tc.tile_pool(name="gw", bufs=8))
        pbx = ctx.enter_context(tc.tile_pool(name="bx", bufs=4))
        pi8 = ctx.enter_context(tc.tile_pool(name="i8", bufs=8))
        pac = ctx.enter_context(tc.tile_pool(name="ac", bufs=1))
        p25 = ctx.enter_context(tc.tile_pool(name="t25", bufs=8))
        pm3 = ctx.enter_context(tc.tile_pool(name="m3", bufs=4))
        pyp = ctx.enter_context(tc.tile_pool(name="yp", bufs=8))
        pgy = ctx.enter_context(tc.tile_pool(name="gy", bufs=1))
        ptr4 = ctx.enter_context(tc.tile_pool(name="tr4", bufs=4))
        ptr2 = ctx.enter_context(tc.tile_pool(name="tr2", bufs=4))
        ptr1 = ctx.enter_context(tc.tile_pool(name="tr1", bufs=1))
        ppm = ctx.enter_context(tc.tile_pool(name="pm", bufs=5, space="PSUM"))
        ppp = ctx.enter_context(tc.tile_pool(name="pp", bufs=1, space="PSUM"))

        # ---- phase A inputs (fp8 DoubleRow layout, k2-granular DMAs) ----
        gw0_sb = []
        mt_sb = []
        for k2 in range(4):
            t = pi8.tile([128, 2 * 1040], f8, tag="i8", name=f"mtd_{k2}")
            nc.sync.dma_start(t[:], mtd[k2, :, :])
            mt_sb.append(t)
            g = pgw.tile([128, 2 * D], f8, tag="gw", name=f"gw0_{k2}")
            nc.sync.dma_start(g[:], gw0d[k2, :, :])
            gw0_sb.append(g)

        # ---- constants into SBUF ----
        def cload(dram, shape, dt):
            t = pconst.tile(shape, dt, tag=dram.name, name=dram.name + "_sb")
            nc.gpsimd.dma_start(t[:], dram[:])
            return t

        w3s_sb = cload(w3sp, [128, NI], bf)
        l1w_sb = cload(l1wp, [128, 128], bf)
        l2w_sb = cload(l2wp, [128, 1], bf)
        idm_sb = cload(idmp, [128, 128], bf)
        l0wh_sb = cload(l0whp, [128, 1024], bf)
        w2s_sb = pac.tile([128, 2048], bf, tag="w2s")
        nc.gpsimd.dma_start(w2s_sb[:], w2sp[:])
        gb0_sb = cload(gb0c, [128, 8], f32)
        gb1_sb = cload(gb1c, [128, 8], f32)
        lb1_sb = cload(lb1c, [128, NI], f32)
        lb2_sb = cload(lb2c, [128, NI], f32)
        ones_sb = cload(onesr, [1, 128], bf)
        b3r8_sb = cload(b3r8, [1, 128], bf)
        l0b_sb = cload(l0bc, [128, 1], f32)
        l1b_sb = cload(l1bc, [128, 1], f32)
        l2b_sb = cload(l2bc2, [128, 2], f32)
        acc_sb = pconst.tile([128, 8], f32, tag="acc")
        nc.vector.memset(acc_sb[:], 0.0)

        # ---- phase A: h_g = relu(M @ gw0 + gb0), fp8 DoubleRow, 1025 cols ----
        # h_g stored fp8 in DoubleRow layout: tile k2 holds m-tiles (2k2, 2k2+1)
        DR = mybir.MatmulPerfMode.DoubleRow
        hg_sb = []
        for k2 in range(4):
            t = pi8.tile([128, 2 * 1040], f8, tag="i8", name=f"hgd_{k2}")
            hg_sb.append(t)
        for m in range(8):
            for (c0, cw) in CH_P:
                ps = ppm.tile([128, 512], f32, tag="pm")
                for k2 in range(4):
                    nc.tensor.matmul(
                        ps[:, :cw],
                        gw0_sb[k2].rearrange("p (ko m) -> p ko m", ko=2)[
                            :, :, m * 128:(m + 1) * 128],
                        mt_sb[k2].rearrange("p (ko b) -> p ko b", ko=2)[
                            :, :, c0:c0 + cw],
                        start=(k2 == 0), stop=(k2 == 3), perf_mode=DR,
                    )
                nc.scalar.activation(
                    hg_sb[m // 2][:, (m % 2) * 1040 + c0:(m % 2) * 1040 + c0 + cw],
                    ps[:, :cw], AF.Relu,
                    bias=gb0_sb[:, m:m + 1], scale=1.0 / WSC,
                )

        # prefetch gw1 (k2-granular)
        gw1_sb = []
        for k2 in range(4):
            g = pgw.tile([128, 2 * D], f8, tag="gw", name=f"gw1_{k2}")
            nc.sync.dma_start(g[:], gw1d[k2, :, :])
            gw1_sb.append(g)

        # ---- phase B: hM = h_g @ gw1 + gb1 (no relu), 1025 cols ----
        hm_sb = []
        for m in range(8):
            t = p25.tile([128, BSP], bf, tag="t25", name=f"hm_{m}")
            hm_sb.append(t)
        for m in range(8):
            for (c0, cw) in CH_P:
                ps = ppm.tile([128, 512], f32, tag="pm")
                for k2 in range(4):
                    nc.tensor.matmul(
                        ps[:, :cw],
                        gw1_sb[k2].rearrange("p (ko m) -> p ko m", ko=2)[
                            :, :, m * 128:(m + 1) * 128],
                        hg_sb[k2].rearrange("p (ko b) -> p ko b", ko=2)[
                            :, :, c0:c0 + cw],
                        start=(k2 == 0), stop=(k2 == 3), perf_mode=DR,
                    )
                nc.scalar.activation(
                    hm_sb[m][:, c0:c0 + cw], ps[:, :cw], AF.Identity,
                    bias=gb1_sb[:, m:m + 1], scale=1.0 / WSC,
                )

        # prefetch phase C inputs (fp8 DoubleRow, k2-granular)
        bx_sb = []
        for k2 in range(4):
            t = pbx.tile([128, 2 * 2176], f8, tag="bx", name=f"bxd_{k2}")
            nc.sync.dma_start(t[:], bxd[k2, :, :])
            bx_sb.append(t)
        yt_sb = []
        for k2 in range(4):
            t = pi8.tile([128, 2 * 1040], f8, tag="i8", name=f"ytd_{k2}")
            nc.sync.dma_start(t[:], ytd[k2, :, :])
            yt_sb.append(t)
        acat_sb = pac.tile([128, 2048], bf, tag="acat")
        nc.gpsimd.dma_start(acat_sb[:], acatp[:])

        # ---- phase C: y_part (m 0..15) and gy (m 16), 1024 cols ----
        # yp[m] = (y @ Bcat)[:, m-block]^T ; gy = y @ l0w[:1024] + l0b (fp32)
        gy_sb = pgy.tile([128, BS], f32, tag="gy")
        yp_sb = [None] * 16

        def emit_C_m(m):
            if m < 16:
                yp_sb[m] = pyp.tile([128, BS], bf, tag="yp", name=f"yp_{m}")
            for (c0, cw) in CH_C:
                ps = ppm.tile([128, 512], f32, tag="pm")
                for k2 in range(4):
                    nc.tensor.matmul(
                        ps[:, :cw],
                        bx_sb[k2].rearrange("p (ko m) -> p ko m", ko=2)[
                            :, :, m * 128:(m + 1) * 128],
                        yt_sb[k2].rearrange("p (ko b) -> p ko b", ko=2)[
                            :, :, c0:c0 + cw],
                        start=(k2 == 0), stop=(k2 == 3), perf_mode=DR,
                    )
                if m < 16:
                    nc.vector.tensor_scalar_mul(
                        yp_sb[m][:, c0:c0 + cw], ps[:, :cw], 1.0 / WSC)
                else:
                    nc.scalar.activation(
                        gy_sb[:, c0:c0 + cw], ps[:, :cw], AF.Identity,
                        bias=l0b_sb[:, 0:1], scale=1.0 / WSC,
                    )

        emit_C_m(16)  # gy first (needed only in F, but frees nothing later)
        for m in range(4):
            emit_C_m(m)

        # local scores: psum_p[p][:, e*8+bt] = s(batch bt*128+row, expert e)
        psum_p = [ppp.tile([128, 128], f32, tag=f"pp{p}", name=f"psum_p{p}")
                  for p in range(2)]
        psum_g = ppp.tile([128, 16], f32, tag="pg", name="psum_g")

        # ---- phase F: global discriminator, both passes ----
        for p in range(2):
            off = p
            sgn = -1.0 if p == 0 else 1.0
            for ci, (c0, cw) in enumerate(CH_C):
                ps = ppm.tile([128, 512], f32, tag="pm")
                for k in range(8):
                    nc.tensor.matmul(
                        ps[:, :cw],
                        l0wh_sb[:, k * 128:(k + 1) * 128],
                        hm_sb[k][:, off + c0:off + c0 + cw],
                        start=(k == 0), stop=(k == 7),
                    )
                z0 = ptr2.tile([128, 512], bf, tag="z0")
                nc.vector.scalar_tensor_tensor(
                    z0[:, :cw], ps[:, :cw], 0.0, gy_sb[:, c0:c0 + cw],
                    op0=OP.add, op1=OP.add)
                h0 = ptr2.tile([128, 512], bf, tag="h0")
                nc.scalar.activation(h0[:, :cw], z0[:, :cw], AF.Relu)
                ps1 = ppm.tile([128, 512], f32, tag="pm")
                nc.tensor.matmul(
                    ps1[:, :cw], l1w_sb[:], h0[:, :cw], start=True, stop=True)
                h1g = ptr2.tile([128, 512], bf, tag="h1g")
                nc.scalar.activation(
                    h1g[:, :cw], ps1[:, :cw], AF.Relu, bias=l1b_sb[:, 0:1])
                for bti in range(4):
                    bt = ci * 4 + bti
                    nc.tensor.matmul(
                        psum_g[:, p * 8 + bt:p * 8 + bt + 1],
                        h1g[:, bti * 128:(bti + 1) * 128],
                        l2w_sb[:, 0:1],
                        start=True, stop=True,
                    )

        # ---- expert phase: z1 = yp + mA (+b1) built in PSUM ----
        # psum := I.T @ yp_chunk  (seed)  +  A_e.T @ M3_chunk ; ACT evicts
        # relu(psum + b1) -> h1; L2 on PE; DVE evicts relu(z2+b2) -> h2;
        # transposed L3 into psum_p columns.
        m3_sb = [None] * 8
        for t in range(2):
            m3t = pm3.tile([128, BSP], bf, tag="m3", name=f"m3_{t}")
            nc.sync.dma_start(m3t[:], m3p[t, :, :])
            m3_sb[t] = m3t
        for e in range(NI):
            # emit C for expert e+4 lazily: keeps the yp pool small and
            # overlaps the remaining y_part matmuls with the expert phase
            if e + 4 < 16 and yp_sb[e + 4] is None:
                emit_C_m(e + 4)

            t = e // 2
            po = 64 * (e % 2)
            if m3_sb[t] is None:
                m3t = pm3.tile([128, BSP], bf, tag="m3", name=f"m3_{t}")
                nc.sync.dma_start(m3t[:], m3p[t, :, :])
                m3_sb[t] = m3t

            for p in range(2):
                off = p  # joint reads cols 0..1023, marginal cols 1..1024
                h1 = ptr4.tile([128, BS], bf, tag="h1", name=f"h1_{e}_{p}")
                for ci, (c0, cw) in enumerate(CH_C):
                    ps = ppm.tile([128, 512], f32, tag="pm")
                    nc.tensor.matmul(
                        ps[:, :cw], idm_sb[:],
                        yp_sb[e][:, c0:c0 + cw],
                        start=True, stop=False,
                    )
                    nc.tensor.matmul(
                        ps[:, :cw],
                        acat_sb[po:po + 64, e * 128:(e + 1) * 128],
                        m3_sb[t][po:po + 64, off + c0:off + c0 + cw],
                        start=False, stop=True,
                    )
                    nc.scalar.activation(
                        h1[:, c0:c0 + cw], ps[:, :cw], AF.Relu,
                        bias=lb1_sb[:, e:e + 1])
                h2 = ptr2.tile([128, BS], bf, tag="h2", name=f"h2_{e}_{p}")
                for ci, (c0, cw) in enumerate(CH_C):
                    ps2 = ppm.tile([128, 512], f32, tag="pm")
                    nc.tensor.matmul(
                        ps2[:, :cw],
                        w2s_sb[:, e * 128:(e + 1) * 128],
                        h1[:, c0:c0 + cw],
                        start=True, stop=True,
                    )
                    nc.vector.tensor_scalar(
                        h2[:, c0:c0 + cw], ps2[:, :cw],
                        lb2_sb[:, e:e + 1], 0.0, op0=OP.add, op1=OP.max)
                # L3 transposed: h2 b-tile stationary, w3 col moving;
                # score for (expert e, batch tile bt) -> psum_p col e*8+bt.
                # b3[e] seeded by a K=1 rank-1 matmul (ones x b3r8).
                nc.tensor.matmul(
                    psum_p[p][:, e * 8:(e + 1) * 8],
                    ones_sb[0:1, :],
                    b3r8_sb[0:1, e * 8:(e + 1) * 8],
                    start=True, stop=False, skip_group_check=True,
                )
                for bt in range(8):
                    nc.tensor.matmul(
                        psum_p[p][:, e * 8 + bt:e * 8 + bt + 1],
                        h2[:, bt * 128:(bt + 1) * 128],
                        w3s_sb[:, e:e + 1],
                        start=False, stop=True, skip_group_check=True,
                    )

        # ---- local softplus reduction: acc col p ----
        for p in range(2):
            sgn = -1.0 if p == 0 else 1.0
            exl = ptr1.tile([128, 128], f32, tag="exl", name=f"exl{p}")
            nc.scalar.activation(exl[:], psum_p[p][:], AF.Exp, scale=sgn)
            spl = ptr1.tile([128, 128], f32, tag="spl", name=f"spl{p}")
            nc.scalar.activation(
                spl[:], exl[:], AF.Ln, bias=1.0,
                accum_out=acc_sb[:, p:p + 1])

        # ---- global softplus reduction: acc col 2+p ----
        for p in range(2):
            sgn = -1.0 if p == 0 else 1.0
            exg = ptr1.tile([128, 16], f32, tag="exg", name=f"exg{p}")
            nc.scalar.activation(
                exg[:, :8], psum_g[:, p * 8:(p + 1) * 8], AF.Exp,
                scale=sgn, bias=l2b_sb[:, p:p + 1])
            spg = ptr1.tile([128, 16], f32, tag="spg", name=f"spg{p}")
            nc.scalar.activation(
                spg[:, :8], exg[:, :8], AF.Ln, bias=1.0,
                accum_out=acc_sb[:, 2 + p:3 + p])

        # ---- output ----
        nc.sync.dma_start(acc[:], acc_sb[:])

    nc.finalize()
    return nc


def _prep_shared(inputs):
    """Weight repack (identical for all cores), fp32 -> bf16."""
    f32 = np.float32
    gw0 = np.asarray(inputs["gw0"], f32)
    gw1 = np.asarray(inputs["gw1"], f32)
    l0w = np.asarray(inputs["l0w"], f32)
    l1w = np.asarray(inputs["l1w"], f32)
    l2w = np.asarray(inputs["l2w"], f32)
    lW1 = np.asarray(inputs["lW1"], f32)
    lW2 = np.asarray(inputs["lW2"], f32)
    lW3 = np.asarray(inputs["lW3"], f32)
    gb0 = np.asarray(inputs["gb0"], f32)
    gb1 = np.asarray(inputs["gb1"], f32)
    l0b = np.asarray(inputs["l0b"], f32)
    l1b = np.asarray(inputs["l1b"], f32)
    l2b = np.asarray(inputs["l2b"], f32)
    lb1 = np.asarray(inputs["lb1"], f32)
    lb2 = np.asarray(inputs["lb2"], f32)
    lb3 = np.asarray(inputs["lb3"], f32)

    def pk(a, kb):  # [K, N] -> [128, (K/128)*N] col-block k = rows k*128..
        K, N = a.shape
        return np.ascontiguousarray(
            a.reshape(K // 128, 128, N).transpose(1, 0, 2).reshape(128, -1))

    def dbl(a, scale=1.0, pad=None):
        # [1024, N] -> [4, 128, 2*Np] fp8 DoubleRow: f = k2*256 + ko*128 + ki
        K, N = a.shape
        Np = N if pad is None else pad
        out = np.zeros((4, 2, 128, Np), np.float32)
        out[:, :, :, :N] = a.reshape(4, 2, 128, N) * scale
        out = out.transpose(0, 2, 1, 3).reshape(4, 128, 2 * Np)
        return np.clip(out, -240.0, 240.0).astype(F8)

    bcatx = np.concatenate(
        [lW1[:, DN:, :].transpose(1, 0, 2).reshape(D, NI * 128), l0w[:D]], axis=1)
    sh = {
        "gw0d": dbl(gw0, WSC),
        "gw1d": dbl(gw1, WSC),
        "bxd": dbl(bcatx, WSC, pad=2176),
        "acatp": np.ascontiguousarray(np.concatenate([
            lW1[:, :DN, :].transpose(1, 0, 2).reshape(DN, NI * 128)] * 2,
            axis=0)).astype(BF),
        "w2sp": np.ascontiguousarray(
            lW2.transpose(1, 0, 2).reshape(128, NI * 128)).astype(BF),
        "w3sp": np.ascontiguousarray(lW3[:, :, 0].T).astype(BF),
        "l0whp": pk(l0w[D:], 128).astype(BF),
        "l1wp": l1w.astype(BF),
        "l2wp": l2w.astype(BF),
        "idmp": np.eye(128, dtype=np.float32).astype(BF),
        "gb0c": np.ascontiguousarray(gb0.reshape(8, 128).T),
        "gb1c": np.ascontiguousarray(gb1.reshape(8, 128).T),
        "lb1c": np.ascontiguousarray(lb1.T),
        "lb2c": np.ascontiguousarray(lb2.T),
        "onesr": np.ones((1, 128), BF),
        "b3r8": np.repeat(lb3[:, 0], 8)[None, :].astype(BF),
        "l0bc": np.ascontiguousarray(l0b[:, None]),
        "l1bc": np.ascontiguousarray(l1b[:, None]),
        "l2bc2": np.ascontiguousarray(
            np.stack([np.full(128, -l2b[0], f32),
                      np.full(128, l2b[0], f32)], axis=1)),
    }
    return sh


def _prep_core(inputs, c):
    f32 = np.float32
    y = np.asarray(inputs["y"], f32)
    M = np.asarray(inputs["M"], f32)
    r0 = c * BS
    rows = np.arange(r0, r0 + BSP) % B  # 1025 rows incl. overlap
    Ms = M[rows]  # [1025, 1024]
    ys = y[r0:r0 + BS]  # [1024, 1024]
    # expert-major M: m3t[e, p, b] = Ms[b, p*16+e]; packed 2 experts/tile
    m3t = np.ascontiguousarray(
        Ms.reshape(BSP, DN, NI).transpose(2, 1, 0)).astype(BF)  # [16,64,1025]

    def dbl8(aT, pad):  # [1024 feat, N] -> [4, 128, 2*pad] fp8
        K, N = aT.shape
        out = np.zeros((4, 2, 128, pad), np.float32)
        out[:, :, :, :N] = aT.reshape(4, 2, 128, N)
        out = out.transpose(0, 2, 1, 3).reshape(4, 128, 2 * pad)
        return np.clip(out, -240.0, 240.0).astype(F8)

    return {
        "ytd": dbl8(ys.T, 1040),
        "mtd": dbl8(Ms.T, 1040),
        "m3p": np.ascontiguousarray(m3t.reshape(8, 128, BSP)),
    }


def combine_partials(accs):
    """accs: list of 8 [128, 8] fp32 arrays -> scalar loss (float64 math)."""
    a = np.stack([np.asarray(x, np.float64) for x in accs])  # [8,128,8]
    sl_j = a[:, :, 0].sum()
    sl_m = a[:, :, 1].sum()
    sg_j = a[:, :, 2].sum()
    sg_m = a[:, :, 3].sum()
    local = BETA * (sl_m + sl_j) / (B * NI)
    glob = ALPHA * (sg_m + sg_j) / B
    return np.float32(local + glob)


def make_in_maps(inputs):
    sh = _prep_shared(inputs)
    return [dict(sh, **_prep_core(inputs, c)) for c in range(NC)]


def get_runner():
    global _RUNNER
    if _RUNNER is None:
        _RUNNER = _build_nc()
    return _RUNNER


def kernel(**inputs) -> np.ndarray:
    from concourse.bass_utils import run_bass_kernel_spmd

    nc = get_runner()
    in_maps = make_in_maps(inputs)
    res = run_bass_kernel_spmd(nc, in_maps, list(range(NC)))
    return combine_partials([r["acc"] for r in res.results])
